# revision 76
# baseline (speedup 1.0000x reference)
"""Multi-head attention (B=2, S=2048, H=1024, 16 heads x 64) on 8 NeuronCores.

Sharding: tensor-parallel over heads x data-parallel over batch.
Core c handles batch (c // 4) and heads [4*(c%4), 4*(c%4)+4).
Each core computes its 4 heads' QKV projections, attention, and the partial
output projection ctx_h @ Wo_h; the host sums the 4 partials per batch.

The datapath is fp16 (noise ~5e-4; fp8 was tried and its ~2.5%/stage
quantization noise transfers 1:1 through the softmax-weighted mean, far
over the accuracy budget). fp16 matmuls run at the same 1 cycle/row as
fp32r but with half the SBUF/DMA traffic. Structural savings vs the fp32
baseline:
 - V is computed directly in [t, dv] layout by making X the stationary
   matmul operand, eliminating all PE transposes and their drains.
 - The output projection packs the two heads of a pair on the contraction
   dim (K=128 instead of 64), halving its PE time. For the first half of
   the sequence it runs as two passes overlapped with late attention
   (pair 0 during h2/h3, pair 1 as h3 filler); the second half runs
   single-pass at the end with drains alternating DVE/ACT.
 - exp outputs fp16 directly (with a -4 global shift so e^score stays in
   range; the shift cancels in the softmax ratio), halving e-tile traffic.
Softmax skips max-subtraction and gets its denominator for free from an
appended ones-column on V; 1/den is broadcast over dv rows with a K=1 PE
outer product (no DRAM round trip). ctx runs 3 t-tiles behind exp so the
in-order PE never waits on ACT latency; projections drip in as
single-matmul filler sub-tasks whose emission order respects each
consumer's deadline (the tile framework only syncs in emission order).
"""
import numpy as np

import concourse.bass as bass
import concourse.tile as tile
from concourse import bacc, mybir
from concourse.bass_utils import run_bass_kernel_spmd

F32 = mybir.dt.float32
F32R = mybir.dt.float32r
F16 = mybir.dt.float16

H, NH, HD = 1024, 16, 64
B, S = 2, 2048
P = 128
NCORES = 8
NHL = 4          # heads per core
DQ = NHL * HD    # 256 projection cols per core
NHT = H // P     # 8 h-tiles
NST = S // P     # 16 t-tiles (also s-tiles)
SB = 512         # matmul free-dim block
SS = 1024        # attention s-superblock (2 PSUM banks)
NSB = S // SB    # 4
NSS = S // SS    # 2

EXP_SHIFT = -4.0  # global exp shift (cancels in softmax); keeps e^score
                  # well inside fp16 range for scores up to ~14


def _reshape_free(ap, dims):
    """Reinterpret a contiguous free region of `ap` as `dims`."""
    total = 1
    new = []
    for d in reversed(dims):
        new.append([total, d])
        total *= d
    assert total == ap.free_size()
    return bass.AP(tensor=ap.tensor, offset=ap.offset,
                   ap=[ap.ap[0]] + list(reversed(new)))


def build_program(repeat=1):
    nc = bacc.Bacc("TRN2", target_bir_lowering=False, debug=False,
                   num_devices=NCORES)
    _lp = nc.allow_low_precision(reason="fp16 attention pipeline")
    _lp.__enter__()

    xt_d = nc.dram_tensor("xt", [H, S], F16, kind="ExternalInput").ap()
    wq_d = nc.dram_tensor("wq", [H, DQ], F16, kind="ExternalInput").ap()
    wk_d = nc.dram_tensor("wk", [H, DQ], F16, kind="ExternalInput").ap()
    wv_d = nc.dram_tensor("wv", [H, DQ], F16, kind="ExternalInput").ap()
    wo_d = nc.dram_tensor("wo", [P, 2, H], F32R, kind="ExternalInput").ap()
    bq_d = nc.dram_tensor("bq", [P, 2], F32, kind="ExternalInput").ap()
    bk_d = nc.dram_tensor("bk", [P, 2], F32, kind="ExternalInput").ap()
    bvb_d = nc.dram_tensor("bvb", [P, DQ], F32, kind="ExternalInput").ap()
    mb_d = nc.dram_tensor("maskb", [P, NST], F32, kind="ExternalInput").ap()
    part_d = nc.dram_tensor("part", [S, H], F16, kind="ExternalOutput").ap()

    with tile.TileContext(nc) as tc:
        with tc.tile_pool(name="big", bufs=1) as big, \
             tc.tile_pool(name="consts", bufs=1) as consts, \
             tc.tile_pool(name="epool", bufs=5) as epool, \
             tc.tile_pool(name="bcpool", bufs=2) as bcpool, \
             tc.tile_pool(name="opool", bufs=1) as opool, \
             tc.tile_pool(name="dpool", bufs=2) as dpool, \
             tc.tile_pool(name="ps_sc", bufs=2, space="PSUM") as ps_sc, \
             tc.tile_pool(name="ps_ctx", bufs=1, space="PSUM") as ps_ctx, \
             tc.tile_pool(name="ps_mm", bufs=2, space="PSUM") as ps_mm:

            for _it in range(repeat):
                # ---------------- input loads ----------------
                xt_sb = big.tile([P, NHT, S], F16, tag="xt", name="xt_sb")
                xt_r = xt_d.rearrange("(n p) s -> n p s", p=P)
                wq_sb = consts.tile([P, NHT, DQ], F16, tag="wq", name="wq_sb")
                wk_sb = consts.tile([P, NHT, DQ], F16, tag="wk", name="wk_sb")
                wv_sb = consts.tile([P, NHT, DQ], F16, tag="wv", name="wv_sb")

                xt_rp = xt_d.rearrange("(n p) s -> p n s", p=P)

                def load_x_cols(c0, c1):
                    nc.sync.dma_start(
                        out=xt_sb[:, :, c0:c1], in_=xt_rp[:, :, c0:c1])

                def load_w(w_sb, w_d):
                    nc.sync.dma_start(
                        out=w_sb, in_=w_d.rearrange("(n p) d -> p n d", p=P))

                load_w(wk_sb, wk_d)
                load_x_cols(0, 256)
                load_w(wq_sb, wq_d)
                load_x_cols(256, 512)
                load_w(wv_sb, wv_d)
                load_x_cols(512, 1024)
                load_x_cols(1024, 1536)
                load_x_cols(1536, 2048)

                bq_sb = consts.tile([P, 2], F32, tag="bq", name="bq_sb")
                bk_sb = consts.tile([P, 2], F32, tag="bk", name="bk_sb")
                nc.sync.dma_start(out=bq_sb, in_=bq_d)
                nc.sync.dma_start(out=bk_sb, in_=bk_d)
                bvb_sb = consts.tile([P, DQ], F32, tag="bvb", name="bvb_sb")
                nc.sync.dma_start(out=bvb_sb, in_=bvb_d)
                mb_sb = consts.tile([P, NST], F32, tag="mb", name="mb_sb")
                nc.sync.dma_start(out=mb_sb, in_=mb_d)
                wo_sb = consts.tile([P, 2, H], F32R, tag="wo", name="wo_sb")
                nc.sync.dma_start(out=wo_sb, in_=wo_d)

                # projection outputs: Q^T/K^T in [dv(2 heads), pair, s]
                qT = big.tile([P, 2, S], F16, tag="qT", name="qT")
                kT = big.tile([P, 2, S], F16, tag="kT", name="kT")
                # V (+ones col) in [t, st, head, dv] layout
                vaug = big.tile([P, NST, NHL, HD + 1], F16, tag="vaug",
                                name="vaug")
                nc.vector.memset(vaug[:, :, :, HD:HD + 1], 1.0)

                ctx2 = [big.tile([P, S], F32R, tag=f"ctx2_{pr}",
                                 name=f"ctx2_{pr}") for pr in range(2)]

                rec_rows = {}
                ones128 = consts.tile([1, P], F32R, tag="ones128",
                                      name="ones128")
                one = nc.const_aps.aps[(F32, 1.0)]
                ones_src = bass.AP(tensor=one.tensor, offset=one.offset,
                                   ap=[[one.ap[0][0], 1], [0, P]])
                nc.vector.tensor_copy(ones128, ones_src)

                # ---------------- projection tasks ----------------
                # emitted as single-matmul sub-tasks (~0.2us each) so filler
                # pops never stall the exp-paced attention pipeline
                def qk_subs(dqt, projs="qk", sbs=tuple(range(NSB))):
                    sel = {"q": (wq_sb, bq_sb, qT, "q"),
                           "k": (wk_sb, bk_sb, kT, "k")}
                    subs = []
                    for sb_i in sbs:
                        for w_sb, b_sb, out_sb, nm in (sel[p] for p in projs):
                            st8 = {}

                            def mm(ht, w_sb=w_sb, sb_i=sb_i, st8=st8, nm=nm):
                                def t():
                                    if ht == 0:
                                        st8["acc"] = ps_mm.tile(
                                            [P, SB], F32, tag="mm512",
                                            name=f"acc_{nm}{dqt}_{sb_i}")
                                    nc.tensor.matmul(
                                        st8["acc"],
                                        w_sb[:, ht, dqt * P:(dqt + 1) * P],
                                        xt_sb[:, ht,
                                              sb_i * SB:(sb_i + 1) * SB],
                                        start=(ht == 0), stop=(ht == NHT - 1))
                                return t

                            def drain(b_sb=b_sb, out_sb=out_sb, sb_i=sb_i,
                                      st8=st8):
                                def t():
                                    nc.vector.tensor_scalar_add(
                                        out_sb[:, dqt,
                                               sb_i * SB:(sb_i + 1) * SB],
                                        st8["acc"], b_sb[:, dqt:dqt + 1])
                                return t

                            subs += [mm(ht) for ht in range(NHT)]
                            subs.append(drain())
                    return subs

                def v_subs(dqt, sts=tuple(range(NST))):
                    subs = []
                    for st in sts:
                        st8 = {}

                        def mm(ht, st=st, st8=st8):
                            def t():
                                if ht == 0:
                                    st8["acc"] = ps_mm.tile(
                                        [P, SB], F32, tag="mm512",
                                        name=f"vacc{dqt}_{st}")
                                nc.tensor.matmul(
                                    st8["acc"][:, 0:P],
                                    xt_sb[:, ht, st * P:(st + 1) * P],
                                    wv_sb[:, ht, dqt * P:(dqt + 1) * P],
                                    start=(ht == 0), stop=(ht == NHT - 1))
                            return t

                        def drain(st=st, st8=st8):
                            def t():
                                nc.vector.tensor_add(
                                    vaug[:, st, 2 * dqt:2 * dqt + 2, 0:HD],
                                    _reshape_free(st8["acc"][:, 0:P], [2, HD]),
                                    _reshape_free(
                                        bvb_sb[:, dqt * P:(dqt + 1) * P],
                                        [2, HD]))
                            return t

                        subs += [mm(ht) for ht in range(NHT)]
                        subs.append(drain())
                    return subs

                # ---------------- attention ----------------
                def attention(h, filler, rate=2.0, mid=None):
                    base = HD * (h % 2)
                    dvt = h // 2
                    pr = h // 2
                    row = HD * (h % 2)
                    budget = 0.0
                    rates = rate if isinstance(rate, tuple) else (rate, rate)
                    for ssb in range(NSS):
                        rate = rates[ssb]
                        if ssb == 1 and mid is not None:
                            mid()
                        acc = ps_ctx.tile([HD + 1, SS], F32, tag="ctxps",
                                          name=f"ctx_{h}_{ssb}")
                        es = {}
                        # ctx runs TWO t-tiles behind exp so the PE (in-order)
                        # never waits on the ACT exp latency or its semaphore
                        LAG = 3
                        for tt in range(NST + LAG):
                            budget += rate
                            while filler and budget >= 1.0:
                                filler.pop(0)()
                                budget -= 1.0
                            if tt < NST:
                                sc = ps_sc.tile([P, SS], F32, tag="sc",
                                                name=f"sc_{h}_{ssb}_{tt}")
                                for half in range(2):
                                    sb_i = 2 * ssb + half
                                    nc.tensor.matmul(
                                        sc[:, half * SB:(half + 1) * SB],
                                        kT[base:base + HD, dvt,
                                           tt * P:(tt + 1) * P],
                                        qT[base:base + HD, dvt,
                                           sb_i * SB:(sb_i + 1) * SB],
                                        start=True, stop=True)
                            if tt >= LAG:
                                e_in = es.pop(tt - LAG)
                                for half in range(2):
                                    nc.tensor.matmul(
                                        acc[:, half * SB:(half + 1) * SB],
                                        vaug[:, tt - LAG, h, :],
                                        e_in[:, half * SB:(half + 1) * SB],
                                        start=(tt == LAG),
                                        stop=(tt == NST + LAG - 1))
                            if tt < NST:
                                e = epool.tile([P, SS], F16, tag="e",
                                               name=f"e_{h}_{ssb}_{tt}")
                                nc.scalar.activation(
                                    out=e, in_=sc,
                                    func=mybir.ActivationFunctionType.Exp,
                                    bias=mb_sb[:, tt:tt + 1], scale=1.0 / 8.0)
                                es[tt] = e
                        # drain ctx + denominator
                        for half in range(2):
                            sb_i = 2 * ssb + half
                            nc.vector.tensor_copy(
                                ctx2[pr][row:row + HD,
                                         sb_i * SB:(sb_i + 1) * SB],
                                acc[0:HD, half * SB:(half + 1) * SB])
                        # reciprocal of the denominator row, in place on-chip
                        rec_row = dpool.tile([1, SS], F32R, tag="recrow",
                                             name=f"recrow_{h}_{ssb}")
                        nc.vector.reciprocal(rec_row, acc[HD:HD + 1, :])
                        rec_rows[(h, ssb)] = rec_row

                def rec_chain(h, ssbs=(0, 1)):
                    # broadcast 1/den over the dv rows with a K=1 PE outer
                    # product (ones64 x rec_row) and scale ctx2 in place --
                    # fully on-chip, no DRAM round trip
                    pr = h // 2
                    row = HD * (h % 2)
                    for ssb in ssbs:
                        rr = rec_rows[(h, ssb)]
                        for half in range(2):
                            sb_i = 2 * ssb + half
                            bc = ps_mm.tile([P, SB], F32, tag="mm512",
                                            name=f"bc_{h}_{sb_i}")
                            nc.tensor.matmul(
                                bc, ones128,
                                rr[:, half * SB:(half + 1) * SB],
                                start=True, stop=True)
                            nc.vector.tensor_mul(
                                ctx2[pr][row:row + HD,
                                         sb_i * SB:(sb_i + 1) * SB],
                                ctx2[pr][row:row + HD,
                                         sb_i * SB:(sb_i + 1) * SB],
                                bc[row:row + HD, :])

                # ---------------- output projection ----------------
                o_st = [None] * NST

                def outproj_p0(st, j):
                    def t():
                        if j == 0:
                            o_st[st] = opool.tile([P, H], F16, tag=f"o_{st}",
                                                  name=f"o_{st}")
                        o = o_st[st]
                        po = ps_mm.tile([P, SB], F32, tag="mm512",
                                        name=f"po0_{st}_{j}")
                        nc.tensor.matmul(
                            po,
                            ctx2[0][:, st * P:(st + 1) * P],
                            wo_sb[:, 0, j * SB:(j + 1) * SB],
                            start=True, stop=True)
                        nc.vector.tensor_copy(o[:, j * SB:(j + 1) * SB], po)
                    return t

                def outproj_p1(st):
                    def t():
                        o = o_st[st]
                        for j in range(2):
                            po = ps_mm.tile([P, SB], F32, tag="mm512",
                                            name=f"po1_{st}_{j}")
                            nc.tensor.matmul(
                                po,
                                ctx2[1][:, st * P:(st + 1) * P],
                                wo_sb[:, 1, j * SB:(j + 1) * SB],
                                start=True, stop=True)
                            nc.vector.tensor_add(
                                o[:, j * SB:(j + 1) * SB],
                                po, o[:, j * SB:(j + 1) * SB])
                        nc.sync.dma_start(
                            out=part_d[st * P:(st + 1) * P, :], in_=o)
                    return t

                def outproj(st, use_act):
                    # single pass over both head pairs; at the kernel tail
                    # the drains alternate DVE / ACT so neither paces it
                    def t():
                        o = opool.tile([P, H], F16, tag=f"o_{st}",
                                       name=f"o_{st}")
                        for j in range(2):
                            po = ps_mm.tile([P, SB], F32, tag="mm512",
                                            name=f"po_{st}_{j}")
                            for pr in range(2):
                                nc.tensor.matmul(
                                    po,
                                    ctx2[pr][:, st * P:(st + 1) * P],
                                    wo_sb[:, pr, j * SB:(j + 1) * SB],
                                    start=(pr == 0), stop=(pr == 1))
                            if use_act and j % 2 == 1:
                                nc.scalar.copy(o[:, j * SB:(j + 1) * SB], po)
                            else:
                                nc.vector.tensor_copy(
                                    o[:, j * SB:(j + 1) * SB], po)
                        nc.sync.dma_start(
                            out=part_d[st * P:(st + 1) * P, :], in_=o)
                    return t

                # ---------------- schedule ----------------
                # inline lead: only what h0's first steps strictly need
                # (K0/Q0 for s,t < 512-1024, V pair-0 tiles 0-3); the rest
                # drips as deadline-ordered fillers during h0-ssb0
                for t in (qk_subs(0, "kq", (0,)) + qk_subs(0, "q", (1,))
                          + v_subs(0, (0, 1, 2, 3))):
                    t()
                # deadline-ordered h0-ssb0 fillers at 9 pops/step: K0-sb_i
                # EMITTED by step 4i, v0_st by step st (emission order is
                # what guarantees readers see written tiles)
                fill = (qk_subs(0, "k", (1,)) + v_subs(0, (4, 5))
                        + qk_subs(0, "k", (2,)) + v_subs(0, (6, 7, 8))
                        + qk_subs(0, "k", (3,))
                        + v_subs(0, (9, 10, 11, 12, 13, 14, 15))
                        + qk_subs(0, "q", (2, 3))
                        + v_subs(1)
                        + qk_subs(1, "k") + qk_subs(1, "q", (0, 1)))
                attention(0, fill, rate=(9.0, 3.2))
                rec_chain(0)
                attention(1, fill, rate=3.2)
                while fill:
                    fill.pop(0)()
                rec_chain(1)
                fill2 = qk_subs(1, "q", (2, 3)) + [
                    outproj_p0(st, j) for st in range(NST // 2)
                    for j in range(2)]
                attention(2, fill2, rate=1.2)
                rec_chain(2)

                def h3_mid():
                    # after h3's first superblock: normalize its s<1024 rows,
                    # then finish the first-half output projection as filler
                    while fill2:
                        fill2.pop(0)()
                    rec_chain(3, ssbs=(0,))
                    fill2.extend(outproj_p1(st) for st in range(NST // 2))

                attention(3, fill2, rate=1.0, mid=h3_mid)
                while fill2:
                    fill2.pop(0)()
                rec_chain(3, ssbs=(1,))
                for st in range(NST // 2, NST):
                    outproj(st, True)()

    nc.compile()
    return nc


_CACHE = {}


def _get_program(repeat=1):
    key = repeat
    if key not in _CACHE:
        _CACHE[key] = build_program(repeat)
    return _CACHE[key]


def _make_in_maps(inputs):
    X = np.asarray(inputs["X"], dtype=np.float32)
    mask = np.asarray(inputs["mask"], dtype=np.float32)
    Wq = np.asarray(inputs["Wq"], dtype=np.float32)
    Wk = np.asarray(inputs["Wk"], dtype=np.float32)
    Wv = np.asarray(inputs["Wv"], dtype=np.float32)
    Wo = np.asarray(inputs["Wo"], dtype=np.float32)
    bq = np.asarray(inputs["bq"], dtype=np.float32)
    bk = np.asarray(inputs["bk"], dtype=np.float32)
    bv = np.asarray(inputs["bv"], dtype=np.float32)

    f16 = np.float16
    in_maps = []
    xts = [np.ascontiguousarray(X[b].T).astype(f16) for b in range(B)]
    maskbs = [np.ascontiguousarray(-1e6 * (1.0 - mask[b])) for b in range(B)]
    for c in range(NCORES):
        b = c // 4
        g = c % 4
        cols = slice(g * DQ, (g + 1) * DQ)
        mb2 = (maskbs[b].reshape(NST, P).T + EXP_SHIFT).astype(np.float32)
        wo2 = Wo[cols, :].reshape(2, P, H).transpose(1, 0, 2)
        in_maps.append({
            "xt": xts[b],
            "wq": np.ascontiguousarray(Wq[:, cols]).astype(f16),
            "wk": np.ascontiguousarray(Wk[:, cols]).astype(f16),
            "wv": np.ascontiguousarray(Wv[:, cols]).astype(f16),
            "wo": np.ascontiguousarray(wo2),
            "bq": np.ascontiguousarray(bq[cols].reshape(2, P).T),
            "bk": np.ascontiguousarray(bk[cols].reshape(2, P).T),
            "bvb": np.ascontiguousarray(
                np.tile(bv[cols].reshape(1, DQ), (P, 1))).astype(np.float32),
            "maskb": np.ascontiguousarray(mb2),
        })
    return in_maps


def kernel(X, mask, Wq, bq, Wk, bk, Wv, bv, Wo, bo):
    bo = np.asarray(bo, dtype=np.float32)
    nc = _get_program()
    in_maps = _make_in_maps(dict(X=X, mask=mask, Wq=Wq, bq=bq, Wk=Wk, bk=bk,
                                 Wv=Wv, bv=bv, Wo=Wo, bo=bo))
    res = run_bass_kernel_spmd(nc, in_maps, list(range(NCORES))).results
    out = np.zeros((B, S, H), dtype=np.float32)
    for c in range(NCORES):
        out[c // 4] += res[c]["part"]
    out += bo
    return out


# revision 82
# speedup vs baseline: 1.0377x; 1.0377x over previous
"""Multi-head attention (B=2, S=2048, H=1024, 16 heads x 64) on 8 NeuronCores.

Sharding: tensor-parallel over heads x data-parallel over batch.
Core c handles batch (c // 4) and heads [4*(c%4), 4*(c%4)+4).
Each core computes its 4 heads' QKV projections, attention, and the partial
output projection ctx_h @ Wo_h; the host sums the 4 partials per batch.

The datapath is fp16 (noise ~5e-4; fp8 was tried and its ~2.5%/stage
quantization noise transfers 1:1 through the softmax-weighted mean, far
over the accuracy budget). fp16 matmuls run at the same 1 cycle/row as
fp32r but with half the SBUF/DMA traffic. Structural savings vs the fp32
baseline:
 - V is computed directly in [t, dv] layout by making X the stationary
   matmul operand, eliminating all PE transposes and their drains.
 - The output projection packs the two heads of a pair on the contraction
   dim (K=128 instead of 64), halving its PE time. For the first half of
   the sequence it runs as two passes overlapped with late attention
   (pair 0 during h2/h3, pair 1 as h3 filler); the second half runs
   single-pass at the end with drains alternating DVE/ACT.
 - exp outputs fp16 directly (with a -4 global shift so e^score stays in
   range; the shift cancels in the softmax ratio), halving e-tile traffic.
Softmax skips max-subtraction and gets its denominator for free from an
appended ones-column on V; 1/den is broadcast over dv rows with a K=1 PE
outer product (no DRAM round trip). ctx runs 3 t-tiles behind exp so the
in-order PE never waits on ACT latency; projections drip in as
single-matmul filler sub-tasks whose emission order respects each
consumer's deadline (the tile framework only syncs in emission order).
"""
import numpy as np

import concourse.bass as bass
import concourse.tile as tile
from concourse import bacc, mybir
from concourse.bass_utils import run_bass_kernel_spmd

F32 = mybir.dt.float32
F32R = mybir.dt.float32r
F16 = mybir.dt.float16

H, NH, HD = 1024, 16, 64
B, S = 2, 2048
P = 128
NCORES = 8
NHL = 4          # heads per core
DQ = NHL * HD    # 256 projection cols per core
NHT = H // P     # 8 h-tiles
NST = S // P     # 16 t-tiles (also s-tiles)
SB = 512         # matmul free-dim block
SS = 1024        # attention s-superblock (2 PSUM banks)
NSB = S // SB    # 4
NSS = S // SS    # 2

EXP_SHIFT = -4.0  # global exp shift (cancels in softmax); keeps e^score
                  # well inside fp16 range for scores up to ~14


def _reshape_free(ap, dims):
    """Reinterpret a contiguous free region of `ap` as `dims`."""
    total = 1
    new = []
    for d in reversed(dims):
        new.append([total, d])
        total *= d
    assert total == ap.free_size()
    return bass.AP(tensor=ap.tensor, offset=ap.offset,
                   ap=[ap.ap[0]] + list(reversed(new)))


def build_program(repeat=1):
    nc = bacc.Bacc("TRN2", target_bir_lowering=False, debug=False,
                   num_devices=NCORES)
    _lp = nc.allow_low_precision(reason="fp16 attention pipeline")
    _lp.__enter__()

    xt_d = nc.dram_tensor("xt", [H, S], F16, kind="ExternalInput").ap()
    wq_d = nc.dram_tensor("wq", [H, DQ], F16, kind="ExternalInput").ap()
    wk_d = nc.dram_tensor("wk", [H, DQ], F16, kind="ExternalInput").ap()
    wv_d = nc.dram_tensor("wv", [H, DQ], F16, kind="ExternalInput").ap()
    wo_d = nc.dram_tensor("wo", [P, 2, H], F32R, kind="ExternalInput").ap()
    bq_d = nc.dram_tensor("bq", [P, 2], F32, kind="ExternalInput").ap()
    bk_d = nc.dram_tensor("bk", [P, 2], F32, kind="ExternalInput").ap()
    bvb_d = nc.dram_tensor("bvb", [P, DQ], F32, kind="ExternalInput").ap()
    mb_d = nc.dram_tensor("maskb", [P, NST], F32, kind="ExternalInput").ap()
    part_d = nc.dram_tensor("part", [S, H], F16, kind="ExternalOutput").ap()

    with tile.TileContext(nc) as tc:
        with tc.tile_pool(name="big", bufs=1) as big, \
             tc.tile_pool(name="consts", bufs=1) as consts, \
             tc.tile_pool(name="epool", bufs=5) as epool, \
             tc.tile_pool(name="bcpool", bufs=2) as bcpool, \
             tc.tile_pool(name="opool", bufs=1) as opool, \
             tc.tile_pool(name="dpool", bufs=2) as dpool, \
             tc.tile_pool(name="ps_sc", bufs=2, space="PSUM") as ps_sc, \
             tc.tile_pool(name="ps_ctx", bufs=1, space="PSUM") as ps_ctx, \
             tc.tile_pool(name="ps_mm", bufs=2, space="PSUM") as ps_mm:

            for _it in range(repeat):
                # ---------------- input loads ----------------
                xt_sb = big.tile([P, NHT, S], F16, tag="xt", name="xt_sb")
                xt_r = xt_d.rearrange("(n p) s -> n p s", p=P)
                wq_sb = consts.tile([P, NHT, DQ], F16, tag="wq", name="wq_sb")
                wk_sb = consts.tile([P, NHT, DQ], F16, tag="wk", name="wk_sb")
                wv_sb = consts.tile([P, NHT, DQ], F16, tag="wv", name="wv_sb")

                xt_rp = xt_d.rearrange("(n p) s -> p n s", p=P)

                def load_x_cols(c0, c1):
                    nc.sync.dma_start(
                        out=xt_sb[:, :, c0:c1], in_=xt_rp[:, :, c0:c1])

                def load_w(w_sb, w_d):
                    nc.sync.dma_start(
                        out=w_sb, in_=w_d.rearrange("(n p) d -> p n d", p=P))

                load_w(wk_sb, wk_d)
                load_x_cols(0, 256)
                load_x_cols(256, 512)
                load_w(wq_sb, wq_d)
                load_w(wv_sb, wv_d)
                load_x_cols(512, 1024)
                load_x_cols(1024, 1536)
                load_x_cols(1536, 2048)

                bq_sb = consts.tile([P, 2], F32, tag="bq", name="bq_sb")
                bk_sb = consts.tile([P, 2], F32, tag="bk", name="bk_sb")
                nc.sync.dma_start(out=bq_sb, in_=bq_d)
                nc.sync.dma_start(out=bk_sb, in_=bk_d)
                bvb_sb = consts.tile([P, DQ], F32, tag="bvb", name="bvb_sb")
                nc.sync.dma_start(out=bvb_sb, in_=bvb_d)
                mb_sb = consts.tile([P, NST], F32, tag="mb", name="mb_sb")
                nc.sync.dma_start(out=mb_sb, in_=mb_d)
                wo_sb = consts.tile([P, 2, H], F32R, tag="wo", name="wo_sb")
                nc.sync.dma_start(out=wo_sb, in_=wo_d)

                # projection outputs: Q^T/K^T in [dv(2 heads), pair, s]
                qT = big.tile([P, 2, S], F16, tag="qT", name="qT")
                kT = big.tile([P, 2, S], F16, tag="kT", name="kT")
                # V (+ones col) in [t, st, head, dv] layout
                vaug = big.tile([P, NST, NHL, HD + 1], F16, tag="vaug",
                                name="vaug")
                nc.vector.memset(vaug[:, :, :, HD:HD + 1], 1.0)

                ctx2 = [big.tile([P, S], F32R, tag=f"ctx2_{pr}",
                                 name=f"ctx2_{pr}") for pr in range(2)]

                rec_rows = {}
                ones128 = consts.tile([1, P], F32R, tag="ones128",
                                      name="ones128")
                one = nc.const_aps.aps[(F32, 1.0)]
                ones_src = bass.AP(tensor=one.tensor, offset=one.offset,
                                   ap=[[one.ap[0][0], 1], [0, P]])
                nc.vector.tensor_copy(ones128, ones_src)

                # ---------------- projection tasks ----------------
                # emitted as single-matmul sub-tasks (~0.2us each) so filler
                # pops never stall the exp-paced attention pipeline
                def qk_subs(dqt, projs="qk", sbs=tuple(range(NSB))):
                    sel = {"q": (wq_sb, bq_sb, qT, "q"),
                           "k": (wk_sb, bk_sb, kT, "k")}
                    subs = []
                    for sb_i in sbs:
                        for w_sb, b_sb, out_sb, nm in (sel[p] for p in projs):
                            st8 = {}

                            def mm(ht, w_sb=w_sb, sb_i=sb_i, st8=st8, nm=nm):
                                def t():
                                    if ht == 0:
                                        st8["acc"] = ps_mm.tile(
                                            [P, SB], F32, tag="mm512",
                                            name=f"acc_{nm}{dqt}_{sb_i}")
                                    nc.tensor.matmul(
                                        st8["acc"],
                                        w_sb[:, ht, dqt * P:(dqt + 1) * P],
                                        xt_sb[:, ht,
                                              sb_i * SB:(sb_i + 1) * SB],
                                        start=(ht == 0), stop=(ht == NHT - 1))
                                return t

                            def drain(b_sb=b_sb, out_sb=out_sb, sb_i=sb_i,
                                      st8=st8):
                                def t():
                                    nc.vector.tensor_scalar_add(
                                        out_sb[:, dqt,
                                               sb_i * SB:(sb_i + 1) * SB],
                                        st8["acc"], b_sb[:, dqt:dqt + 1])
                                return t

                            subs += [mm(ht) for ht in range(NHT)]
                            subs.append(drain())
                    return subs

                def v_subs(dqt, sts=tuple(range(NST))):
                    subs = []
                    for st in sts:
                        st8 = {}

                        def mm(ht, st=st, st8=st8):
                            def t():
                                if ht == 0:
                                    st8["acc"] = ps_mm.tile(
                                        [P, SB], F32, tag="mm512",
                                        name=f"vacc{dqt}_{st}")
                                nc.tensor.matmul(
                                    st8["acc"][:, 0:P],
                                    xt_sb[:, ht, st * P:(st + 1) * P],
                                    wv_sb[:, ht, dqt * P:(dqt + 1) * P],
                                    start=(ht == 0), stop=(ht == NHT - 1))
                            return t

                        def drain(st=st, st8=st8):
                            def t():
                                nc.vector.tensor_add(
                                    vaug[:, st, 2 * dqt:2 * dqt + 2, 0:HD],
                                    _reshape_free(st8["acc"][:, 0:P], [2, HD]),
                                    _reshape_free(
                                        bvb_sb[:, dqt * P:(dqt + 1) * P],
                                        [2, HD]))
                            return t

                        subs += [mm(ht) for ht in range(NHT)]
                        subs.append(drain())
                    return subs

                # ---------------- attention ----------------
                def attention(h, filler, rate=2.0, mid=None):
                    base = HD * (h % 2)
                    dvt = h // 2
                    pr = h // 2
                    row = HD * (h % 2)
                    budget = 0.0
                    rates = rate if isinstance(rate, tuple) else (rate, rate)
                    for ssb in range(NSS):
                        rate = rates[ssb]
                        if ssb == 1 and mid is not None:
                            mid()
                        acc = ps_ctx.tile([HD + 1, SS], F32, tag="ctxps",
                                          name=f"ctx_{h}_{ssb}")
                        es = {}
                        # ctx runs TWO t-tiles behind exp so the PE (in-order)
                        # never waits on the ACT exp latency or its semaphore
                        LAG = 3
                        for tt in range(NST + LAG):
                            budget += rate
                            while filler and budget >= 1.0:
                                filler.pop(0)()
                                budget -= 1.0
                            if tt < NST:
                                sc = ps_sc.tile([P, SS], F32, tag="sc",
                                                name=f"sc_{h}_{ssb}_{tt}")
                                for half in range(2):
                                    sb_i = 2 * ssb + half
                                    nc.tensor.matmul(
                                        sc[:, half * SB:(half + 1) * SB],
                                        kT[base:base + HD, dvt,
                                           tt * P:(tt + 1) * P],
                                        qT[base:base + HD, dvt,
                                           sb_i * SB:(sb_i + 1) * SB],
                                        start=True, stop=True)
                            if tt >= LAG:
                                e_in = es.pop(tt - LAG)
                                for half in range(2):
                                    nc.tensor.matmul(
                                        acc[:, half * SB:(half + 1) * SB],
                                        vaug[:, tt - LAG, h, :],
                                        e_in[:, half * SB:(half + 1) * SB],
                                        start=(tt == LAG),
                                        stop=(tt == NST + LAG - 1))
                            if tt < NST:
                                e = epool.tile([P, SS], F16, tag="e",
                                               name=f"e_{h}_{ssb}_{tt}")
                                nc.scalar.activation(
                                    out=e, in_=sc,
                                    func=mybir.ActivationFunctionType.Exp,
                                    bias=mb_sb[:, tt:tt + 1], scale=1.0 / 8.0)
                                es[tt] = e
                        # drain ctx + denominator
                        for half in range(2):
                            sb_i = 2 * ssb + half
                            nc.vector.tensor_copy(
                                ctx2[pr][row:row + HD,
                                         sb_i * SB:(sb_i + 1) * SB],
                                acc[0:HD, half * SB:(half + 1) * SB])
                        # reciprocal of the denominator row, in place on-chip
                        rec_row = dpool.tile([1, SS], F32R, tag="recrow",
                                             name=f"recrow_{h}_{ssb}")
                        # two half-recips so a consumer of the first half
                        # needn't wait for the whole row
                        nc.vector.reciprocal(rec_row[:, 0:SB],
                                             acc[HD:HD + 1, 0:SB])
                        nc.vector.reciprocal(rec_row[:, SB:SS],
                                             acc[HD:HD + 1, SB:SS])
                        rec_rows[(h, ssb)] = rec_row

                def rec_chain(h, ssbs=(0, 1), halves=(0, 1)):
                    # broadcast 1/den over the dv rows with a K=1 PE outer
                    # product (ones128 x rec_row) and scale ctx2 in place --
                    # fully on-chip, no DRAM round trip
                    pr = h // 2
                    row = HD * (h % 2)
                    for ssb in ssbs:
                        rr = rec_rows[(h, ssb)]
                        for half in halves:
                            sb_i = 2 * ssb + half
                            bc = ps_mm.tile([P, SB], F32, tag="mm512",
                                            name=f"bc_{h}_{sb_i}")
                            nc.tensor.matmul(
                                bc, ones128,
                                rr[:, half * SB:(half + 1) * SB],
                                start=True, stop=True)
                            nc.vector.tensor_mul(
                                ctx2[pr][row:row + HD,
                                         sb_i * SB:(sb_i + 1) * SB],
                                ctx2[pr][row:row + HD,
                                         sb_i * SB:(sb_i + 1) * SB],
                                bc[row:row + HD, :])

                # ---------------- output projection ----------------
                o_st = [None] * NST

                def outproj_p0(st, j):
                    def t():
                        if j == 0:
                            o_st[st] = opool.tile([P, H], F16, tag=f"o_{st}",
                                                  name=f"o_{st}")
                        o = o_st[st]
                        po = ps_mm.tile([P, SB], F32, tag="mm512",
                                        name=f"po0_{st}_{j}")
                        nc.tensor.matmul(
                            po,
                            ctx2[0][:, st * P:(st + 1) * P],
                            wo_sb[:, 0, j * SB:(j + 1) * SB],
                            start=True, stop=True)
                        nc.vector.tensor_copy(o[:, j * SB:(j + 1) * SB], po)
                    return t

                def outproj_p1(st):
                    def t():
                        o = o_st[st]
                        for j in range(2):
                            po = ps_mm.tile([P, SB], F32, tag="mm512",
                                            name=f"po1_{st}_{j}")
                            nc.tensor.matmul(
                                po,
                                ctx2[1][:, st * P:(st + 1) * P],
                                wo_sb[:, 1, j * SB:(j + 1) * SB],
                                start=True, stop=True)
                            nc.vector.tensor_add(
                                o[:, j * SB:(j + 1) * SB],
                                po, o[:, j * SB:(j + 1) * SB])
                        nc.sync.dma_start(
                            out=part_d[st * P:(st + 1) * P, :], in_=o)
                    return t

                def outproj(st, use_act):
                    # single pass over both head pairs; at the kernel tail
                    # the drains alternate DVE / ACT so neither paces it,
                    # and po tiles alternate ps_mm / the (now idle) score
                    # pool so PSUM rotation latency doesn't pace it either
                    def t():
                        o = opool.tile([P, H], F16, tag=f"o_{st}",
                                       name=f"o_{st}")
                        for j in range(2):
                            if use_act and j % 2 == 1:
                                po = ps_sc.tile([P, SS], F32, tag="sc",
                                                name=f"po_{st}_{j}")[:, 0:SB]
                            else:
                                po = ps_mm.tile([P, SB], F32, tag="mm512",
                                                name=f"po_{st}_{j}")
                            for pr in range(2):
                                nc.tensor.matmul(
                                    po,
                                    ctx2[pr][:, st * P:(st + 1) * P],
                                    wo_sb[:, pr, j * SB:(j + 1) * SB],
                                    start=(pr == 0), stop=(pr == 1))
                            if use_act and j % 2 == 1:
                                nc.scalar.copy(o[:, j * SB:(j + 1) * SB], po)
                            else:
                                nc.vector.tensor_copy(
                                    o[:, j * SB:(j + 1) * SB], po)
                        nc.sync.dma_start(
                            out=part_d[st * P:(st + 1) * P, :], in_=o)
                    return t

                # ---------------- schedule ----------------
                # inline lead: only what h0's first steps strictly need
                # (K0/Q0 for s,t < 512-1024, V pair-0 tiles 0-3); the rest
                # drips as deadline-ordered fillers during h0-ssb0
                for t in (qk_subs(0, "kq", (0,)) + qk_subs(0, "q", (1,))
                          + v_subs(0, (0, 1, 2, 3))):
                    t()
                # deadline-ordered h0-ssb0 fillers at 9 pops/step: K0-sb_i
                # EMITTED by step 4i, v0_st by step st (emission order is
                # what guarantees readers see written tiles)
                fill = (qk_subs(0, "k", (1,)) + v_subs(0, (4, 5))
                        + qk_subs(0, "k", (2,)) + v_subs(0, (6, 7, 8))
                        + qk_subs(0, "k", (3,))
                        + v_subs(0, (9, 10, 11, 12, 13, 14, 15))
                        + qk_subs(0, "q", (2, 3))
                        + v_subs(1)
                        + qk_subs(1, "k") + qk_subs(1, "q", (0, 1)))
                attention(0, fill, rate=(9.0, 3.2))
                rec_chain(0)
                attention(1, fill, rate=3.2)
                while fill:
                    fill.pop(0)()
                rec_chain(1)
                fill2 = qk_subs(1, "q", (2, 3)) + [
                    outproj_p0(st, j) for st in range(NST // 2)
                    for j in range(2)]
                attention(2, fill2, rate=1.2)
                rec_chain(2)

                def h3_mid():
                    # after h3's first superblock: normalize its s<1024 rows,
                    # then finish the first-half output projection as filler
                    while fill2:
                        fill2.pop(0)()
                    rec_chain(3, ssbs=(0,))
                    fill2.extend(outproj_p1(st) for st in range(NST // 2))

                attention(3, fill2, rate=1.0, mid=h3_mid)
                while fill2:
                    fill2.pop(0)()
                # per-half tail: outproj for s in [1024,1536) starts right
                # after the first half-reciprocal; the second half's
                # normalization overlaps it
                rec_chain(3, ssbs=(1,), halves=(0,))
                outproj(8, True)()
                rec_chain(3, ssbs=(1,), halves=(1,))
                for st in range(9, NST):
                    outproj(st, True)()

    nc.compile()
    return nc


_CACHE = {}


def _get_program(repeat=1):
    key = repeat
    if key not in _CACHE:
        _CACHE[key] = build_program(repeat)
    return _CACHE[key]


def _make_in_maps(inputs):
    X = np.asarray(inputs["X"], dtype=np.float32)
    mask = np.asarray(inputs["mask"], dtype=np.float32)
    Wq = np.asarray(inputs["Wq"], dtype=np.float32)
    Wk = np.asarray(inputs["Wk"], dtype=np.float32)
    Wv = np.asarray(inputs["Wv"], dtype=np.float32)
    Wo = np.asarray(inputs["Wo"], dtype=np.float32)
    bq = np.asarray(inputs["bq"], dtype=np.float32)
    bk = np.asarray(inputs["bk"], dtype=np.float32)
    bv = np.asarray(inputs["bv"], dtype=np.float32)

    f16 = np.float16
    in_maps = []
    xts = [np.ascontiguousarray(X[b].T).astype(f16) for b in range(B)]
    maskbs = [np.ascontiguousarray(-1e6 * (1.0 - mask[b])) for b in range(B)]
    for c in range(NCORES):
        b = c // 4
        g = c % 4
        cols = slice(g * DQ, (g + 1) * DQ)
        mb2 = (maskbs[b].reshape(NST, P).T + EXP_SHIFT).astype(np.float32)
        wo2 = Wo[cols, :].reshape(2, P, H).transpose(1, 0, 2)
        in_maps.append({
            "xt": xts[b],
            "wq": np.ascontiguousarray(Wq[:, cols]).astype(f16),
            "wk": np.ascontiguousarray(Wk[:, cols]).astype(f16),
            "wv": np.ascontiguousarray(Wv[:, cols]).astype(f16),
            "wo": np.ascontiguousarray(wo2),
            "bq": np.ascontiguousarray(bq[cols].reshape(2, P).T),
            "bk": np.ascontiguousarray(bk[cols].reshape(2, P).T),
            "bvb": np.ascontiguousarray(
                np.tile(bv[cols].reshape(1, DQ), (P, 1))).astype(np.float32),
            "maskb": np.ascontiguousarray(mb2),
        })
    return in_maps


def kernel(X, mask, Wq, bq, Wk, bk, Wv, bv, Wo, bo):
    bo = np.asarray(bo, dtype=np.float32)
    nc = _get_program()
    in_maps = _make_in_maps(dict(X=X, mask=mask, Wq=Wq, bq=bq, Wk=Wk, bk=bk,
                                 Wv=Wv, bv=bv, Wo=Wo, bo=bo))
    res = run_bass_kernel_spmd(nc, in_maps, list(range(NCORES))).results
    out = np.zeros((B, S, H), dtype=np.float32)
    for c in range(NCORES):
        out[c // 4] += res[c]["part"]
    out += bo
    return out


# revision 86
# speedup vs baseline: 1.0572x; 1.0188x over previous
"""Multi-head attention (B=2, S=2048, H=1024, 16 heads x 64) on 8 NeuronCores.

Sharding: tensor-parallel over heads x data-parallel over batch.
Core c handles batch (c // 4) and heads [4*(c%4), 4*(c%4)+4).
Each core computes its 4 heads' QKV projections, attention, and the partial
output projection ctx_h @ Wo_h; the host sums the 4 partials per batch.

The datapath is fp16 (noise ~5e-4; fp8 was tried and its ~2.5%/stage
quantization noise transfers 1:1 through the softmax-weighted mean, far
over the accuracy budget). fp16 matmuls run at the same 1 cycle/row as
fp32r but with half the SBUF/DMA traffic. Structural savings vs the fp32
baseline:
 - V is computed directly in [t, dv] layout by making X the stationary
   matmul operand, eliminating all PE transposes and their drains.
 - The output projection packs the two heads of a pair on the contraction
   dim (K=128 instead of 64), halving its PE time. For the first half of
   the sequence it runs as two passes overlapped with late attention
   (pair 0 during h2/h3, pair 1 as h3 filler); the second half runs
   single-pass at the end with drains alternating DVE/ACT.
 - exp outputs fp16 directly (with a -4 global shift so e^score stays in
   range; the shift cancels in the softmax ratio), halving e-tile traffic.
Softmax skips max-subtraction and gets its denominator for free from an
appended ones-column on V; 1/den is broadcast over dv rows with a K=1 PE
outer product (no DRAM round trip). ctx runs 3 t-tiles behind exp so the
in-order PE never waits on ACT latency; projections drip in as
single-matmul filler sub-tasks whose emission order respects each
consumer's deadline (the tile framework only syncs in emission order).
"""
import numpy as np

import concourse.bass as bass
import concourse.tile as tile
from concourse import bacc, mybir
from concourse.bass_utils import run_bass_kernel_spmd

F32 = mybir.dt.float32
F32R = mybir.dt.float32r
F16 = mybir.dt.float16

H, NH, HD = 1024, 16, 64
B, S = 2, 2048
P = 128
NCORES = 8
NHL = 4          # heads per core
DQ = NHL * HD    # 256 projection cols per core
NHT = H // P     # 8 h-tiles
NST = S // P     # 16 t-tiles (also s-tiles)
SB = 512         # matmul free-dim block
SS = 1024        # attention s-superblock (2 PSUM banks)
NSB = S // SB    # 4
NSS = S // SS    # 2

EXP_SHIFT = -4.0  # global exp shift (cancels in softmax); keeps e^score
                  # well inside fp16 range for scores up to ~14


def _reshape_free(ap, dims):
    """Reinterpret a contiguous free region of `ap` as `dims`."""
    total = 1
    new = []
    for d in reversed(dims):
        new.append([total, d])
        total *= d
    assert total == ap.free_size()
    return bass.AP(tensor=ap.tensor, offset=ap.offset,
                   ap=[ap.ap[0]] + list(reversed(new)))


def build_program(repeat=1):
    nc = bacc.Bacc("TRN2", target_bir_lowering=False, debug=False,
                   num_devices=NCORES)
    _lp = nc.allow_low_precision(reason="fp16 attention pipeline")
    _lp.__enter__()

    xt_d = nc.dram_tensor("xt", [H, S], F16, kind="ExternalInput").ap()
    wq_d = nc.dram_tensor("wq", [H, DQ], F16, kind="ExternalInput").ap()
    wk_d = nc.dram_tensor("wk", [H, DQ], F16, kind="ExternalInput").ap()
    wv_d = nc.dram_tensor("wv", [H, DQ], F16, kind="ExternalInput").ap()
    wo_d = nc.dram_tensor("wo", [P, 2, H], F32R, kind="ExternalInput").ap()
    bq_d = nc.dram_tensor("bq", [P, 2], F32, kind="ExternalInput").ap()
    bk_d = nc.dram_tensor("bk", [P, 2], F32, kind="ExternalInput").ap()
    bvb_d = nc.dram_tensor("bvb", [P, DQ], F32, kind="ExternalInput").ap()
    mb_d = nc.dram_tensor("maskb", [P, NST], F32, kind="ExternalInput").ap()
    part_d = nc.dram_tensor("part", [S, H], F16, kind="ExternalOutput").ap()

    with tile.TileContext(nc) as tc:
        with tc.tile_pool(name="big", bufs=1) as big, \
             tc.tile_pool(name="consts", bufs=1) as consts, \
             tc.tile_pool(name="epool", bufs=5) as epool, \
             tc.tile_pool(name="bcpool", bufs=2) as bcpool, \
             tc.tile_pool(name="opool", bufs=1) as opool, \
             tc.tile_pool(name="dpool", bufs=2) as dpool, \
             tc.tile_pool(name="ps_sc", bufs=2, space="PSUM") as ps_sc, \
             tc.tile_pool(name="ps_ctx", bufs=1, space="PSUM") as ps_ctx, \
             tc.tile_pool(name="ps_mm", bufs=2, space="PSUM") as ps_mm:

            for _it in range(repeat):
                # ---------------- input loads ----------------
                xt_sb = big.tile([P, NHT, S], F16, tag="xt", name="xt_sb")
                xt_r = xt_d.rearrange("(n p) s -> n p s", p=P)
                wq_sb = consts.tile([P, NHT, DQ], F16, tag="wq", name="wq_sb")
                wk_sb = consts.tile([P, NHT, DQ], F16, tag="wk", name="wk_sb")
                wv_sb = consts.tile([P, NHT, DQ], F16, tag="wv", name="wv_sb")

                xt_rp = xt_d.rearrange("(n p) s -> p n s", p=P)

                def load_x_cols(c0, c1):
                    nc.sync.dma_start(
                        out=xt_sb[:, :, c0:c1], in_=xt_rp[:, :, c0:c1])

                def load_w(w_sb, w_d):
                    nc.sync.dma_start(
                        out=w_sb, in_=w_d.rearrange("(n p) d -> p n d", p=P))

                load_w(wk_sb, wk_d)
                load_x_cols(0, 256)
                load_x_cols(256, 512)
                load_w(wv_sb, wv_d)
                load_w(wq_sb, wq_d)
                load_x_cols(512, 1024)
                load_x_cols(1024, 1536)
                load_x_cols(1536, 2048)

                # tiny tensors ride the idle gpsimd queue so they land in
                # the first few us instead of behind the X stream (the
                # first exp needs mb, the first drains need bq/bk/bvb)
                bq_sb = consts.tile([P, 2], F32, tag="bq", name="bq_sb")
                bk_sb = consts.tile([P, 2], F32, tag="bk", name="bk_sb")
                nc.gpsimd.dma_start(out=bq_sb, in_=bq_d)
                nc.gpsimd.dma_start(out=bk_sb, in_=bk_d)
                mb_sb = consts.tile([P, NST], F32, tag="mb", name="mb_sb")
                nc.gpsimd.dma_start(out=mb_sb, in_=mb_d)
                bvb_sb = consts.tile([P, DQ], F32, tag="bvb", name="bvb_sb")
                nc.gpsimd.dma_start(out=bvb_sb, in_=bvb_d)
                wo_sb = consts.tile([P, 2, H], F32R, tag="wo", name="wo_sb")
                nc.sync.dma_start(out=wo_sb, in_=wo_d)

                # projection outputs: Q^T/K^T in [dv(2 heads), pair, s]
                qT = big.tile([P, 2, S], F16, tag="qT", name="qT")
                kT = big.tile([P, 2, S], F16, tag="kT", name="kT")
                # V (+ones col) in [t, st, head, dv] layout
                vaug = big.tile([P, NST, NHL, HD + 1], F16, tag="vaug",
                                name="vaug")
                nc.vector.memset(vaug[:, :, :, HD:HD + 1], 1.0)

                ctx2 = [big.tile([P, S], F32R, tag=f"ctx2_{pr}",
                                 name=f"ctx2_{pr}") for pr in range(2)]

                rec_rows = {}
                ones128 = consts.tile([1, P], F32R, tag="ones128",
                                      name="ones128")
                one = nc.const_aps.aps[(F32, 1.0)]
                ones_src = bass.AP(tensor=one.tensor, offset=one.offset,
                                   ap=[[one.ap[0][0], 1], [0, P]])
                nc.vector.tensor_copy(ones128, ones_src)

                # dummy exp to pull the ACT Exp-table load (1.3us) into the
                # DMA-bound lead instead of the first real exp's critical path
                warm = consts.tile([1, 1], F16, tag="warm", name="warm")
                nc.scalar.activation(out=warm, in_=ones128[0:1, 0:1],
                                     func=mybir.ActivationFunctionType.Exp,
                                     bias=0.0, scale=1.0)

                # ---------------- projection tasks ----------------
                # emitted as single-matmul sub-tasks (~0.2us each) so filler
                # pops never stall the exp-paced attention pipeline
                def qk_subs(dqt, projs="qk", sbs=tuple(range(NSB))):
                    sel = {"q": (wq_sb, bq_sb, qT, "q"),
                           "k": (wk_sb, bk_sb, kT, "k")}
                    subs = []
                    for sb_i in sbs:
                        for w_sb, b_sb, out_sb, nm in (sel[p] for p in projs):
                            st8 = {}

                            def mm(ht, w_sb=w_sb, sb_i=sb_i, st8=st8, nm=nm):
                                def t():
                                    if ht == 0:
                                        st8["acc"] = ps_mm.tile(
                                            [P, SB], F32, tag="mm512",
                                            name=f"acc_{nm}{dqt}_{sb_i}")
                                    nc.tensor.matmul(
                                        st8["acc"],
                                        w_sb[:, ht, dqt * P:(dqt + 1) * P],
                                        xt_sb[:, ht,
                                              sb_i * SB:(sb_i + 1) * SB],
                                        start=(ht == 0), stop=(ht == NHT - 1))
                                return t

                            def drain(b_sb=b_sb, out_sb=out_sb, sb_i=sb_i,
                                      st8=st8):
                                def t():
                                    nc.vector.tensor_scalar_add(
                                        out_sb[:, dqt,
                                               sb_i * SB:(sb_i + 1) * SB],
                                        st8["acc"], b_sb[:, dqt:dqt + 1])
                                return t

                            subs += [mm(ht) for ht in range(NHT)]
                            subs.append(drain())
                    return subs

                def v_subs(dqt, sts=tuple(range(NST))):
                    subs = []
                    for st in sts:
                        st8 = {}

                        def mm(ht, st=st, st8=st8):
                            def t():
                                if ht == 0:
                                    st8["acc"] = ps_mm.tile(
                                        [P, SB], F32, tag="mm512",
                                        name=f"vacc{dqt}_{st}")
                                nc.tensor.matmul(
                                    st8["acc"][:, 0:P],
                                    xt_sb[:, ht, st * P:(st + 1) * P],
                                    wv_sb[:, ht, dqt * P:(dqt + 1) * P],
                                    start=(ht == 0), stop=(ht == NHT - 1))
                            return t

                        def drain(st=st, st8=st8):
                            def t():
                                nc.vector.tensor_add(
                                    vaug[:, st, 2 * dqt:2 * dqt + 2, 0:HD],
                                    _reshape_free(st8["acc"][:, 0:P], [2, HD]),
                                    _reshape_free(
                                        bvb_sb[:, dqt * P:(dqt + 1) * P],
                                        [2, HD]))
                            return t

                        subs += [mm(ht) for ht in range(NHT)]
                        subs.append(drain())
                    return subs

                # ---------------- attention ----------------
                def attention(h, filler, rate=2.0, mid=None):
                    base = HD * (h % 2)
                    dvt = h // 2
                    pr = h // 2
                    row = HD * (h % 2)
                    budget = 0.0
                    rates = rate if isinstance(rate, tuple) else (rate, rate)
                    for ssb in range(NSS):
                        rate = rates[ssb]
                        if ssb == 1 and mid is not None:
                            mid()
                        acc = ps_ctx.tile([HD + 1, SS], F32, tag="ctxps",
                                          name=f"ctx_{h}_{ssb}")
                        es = {}
                        # ctx runs TWO t-tiles behind exp so the PE (in-order)
                        # never waits on the ACT exp latency or its semaphore
                        LAG = 3
                        for tt in range(NST + LAG):
                            budget += rate
                            while filler and budget >= 1.0:
                                filler.pop(0)()
                                budget -= 1.0
                            if tt < NST:
                                sc = ps_sc.tile([P, SS], F32, tag="sc",
                                                name=f"sc_{h}_{ssb}_{tt}")
                                for half in range(2):
                                    sb_i = 2 * ssb + half
                                    nc.tensor.matmul(
                                        sc[:, half * SB:(half + 1) * SB],
                                        kT[base:base + HD, dvt,
                                           tt * P:(tt + 1) * P],
                                        qT[base:base + HD, dvt,
                                           sb_i * SB:(sb_i + 1) * SB],
                                        start=True, stop=True)
                            if tt >= LAG:
                                e_in = es.pop(tt - LAG)
                                for half in range(2):
                                    nc.tensor.matmul(
                                        acc[:, half * SB:(half + 1) * SB],
                                        vaug[:, tt - LAG, h, :],
                                        e_in[:, half * SB:(half + 1) * SB],
                                        start=(tt == LAG),
                                        stop=(tt == NST + LAG - 1))
                            if tt < NST:
                                e = epool.tile([P, SS], F16, tag="e",
                                               name=f"e_{h}_{ssb}_{tt}")
                                nc.scalar.activation(
                                    out=e, in_=sc,
                                    func=mybir.ActivationFunctionType.Exp,
                                    bias=mb_sb[:, tt:tt + 1], scale=1.0 / 8.0)
                                es[tt] = e
                        # drain ctx + denominator
                        for half in range(2):
                            sb_i = 2 * ssb + half
                            nc.vector.tensor_copy(
                                ctx2[pr][row:row + HD,
                                         sb_i * SB:(sb_i + 1) * SB],
                                acc[0:HD, half * SB:(half + 1) * SB])
                        # reciprocal of the denominator row, in place on-chip
                        rec_row = dpool.tile([1, SS], F32R, tag="recrow",
                                             name=f"recrow_{h}_{ssb}")
                        # two half-recips so a consumer of the first half
                        # needn't wait for the whole row
                        nc.vector.reciprocal(rec_row[:, 0:SB],
                                             acc[HD:HD + 1, 0:SB])
                        nc.vector.reciprocal(rec_row[:, SB:SS],
                                             acc[HD:HD + 1, SB:SS])
                        rec_rows[(h, ssb)] = rec_row

                def rec_chain(h, ssbs=(0, 1), halves=(0, 1)):
                    # broadcast 1/den over the dv rows with a K=1 PE outer
                    # product (ones128 x rec_row) and scale ctx2 in place --
                    # fully on-chip, no DRAM round trip
                    pr = h // 2
                    row = HD * (h % 2)
                    for ssb in ssbs:
                        rr = rec_rows[(h, ssb)]
                        for half in halves:
                            sb_i = 2 * ssb + half
                            bc = ps_mm.tile([P, SB], F32, tag="mm512",
                                            name=f"bc_{h}_{sb_i}")
                            nc.tensor.matmul(
                                bc, ones128,
                                rr[:, half * SB:(half + 1) * SB],
                                start=True, stop=True)
                            nc.vector.tensor_mul(
                                ctx2[pr][row:row + HD,
                                         sb_i * SB:(sb_i + 1) * SB],
                                ctx2[pr][row:row + HD,
                                         sb_i * SB:(sb_i + 1) * SB],
                                bc[row:row + HD, :])

                # ---------------- output projection ----------------
                o_st = [None] * NST

                def outproj_p0(st, j):
                    def t():
                        if j == 0:
                            o_st[st] = opool.tile([P, H], F16, tag=f"o_{st}",
                                                  name=f"o_{st}")
                        o = o_st[st]
                        po = ps_mm.tile([P, SB], F32, tag="mm512",
                                        name=f"po0_{st}_{j}")
                        nc.tensor.matmul(
                            po,
                            ctx2[0][:, st * P:(st + 1) * P],
                            wo_sb[:, 0, j * SB:(j + 1) * SB],
                            start=True, stop=True)
                        nc.vector.tensor_copy(o[:, j * SB:(j + 1) * SB], po)
                    return t

                def outproj_p1(st):
                    def t():
                        o = o_st[st]
                        for j in range(2):
                            po = ps_mm.tile([P, SB], F32, tag="mm512",
                                            name=f"po1_{st}_{j}")
                            nc.tensor.matmul(
                                po,
                                ctx2[1][:, st * P:(st + 1) * P],
                                wo_sb[:, 1, j * SB:(j + 1) * SB],
                                start=True, stop=True)
                            nc.vector.tensor_add(
                                o[:, j * SB:(j + 1) * SB],
                                po, o[:, j * SB:(j + 1) * SB])
                        nc.sync.dma_start(
                            out=part_d[st * P:(st + 1) * P, :], in_=o)
                    return t

                def outproj(st, use_act):
                    # single pass over both head pairs; at the kernel tail
                    # the drains alternate DVE / ACT so neither paces it,
                    # and po tiles alternate ps_mm / the (now idle) score
                    # pool so PSUM rotation latency doesn't pace it either
                    def t():
                        o = opool.tile([P, H], F16, tag=f"o_{st}",
                                       name=f"o_{st}")
                        for j in range(2):
                            if use_act and j % 2 == 1:
                                po = ps_sc.tile([P, SS], F32, tag="sc",
                                                name=f"po_{st}_{j}")[:, 0:SB]
                            else:
                                po = ps_mm.tile([P, SB], F32, tag="mm512",
                                                name=f"po_{st}_{j}")
                            for pr in range(2):
                                nc.tensor.matmul(
                                    po,
                                    ctx2[pr][:, st * P:(st + 1) * P],
                                    wo_sb[:, pr, j * SB:(j + 1) * SB],
                                    start=(pr == 0), stop=(pr == 1))
                            if use_act and j % 2 == 1:
                                nc.scalar.copy(o[:, j * SB:(j + 1) * SB], po)
                            else:
                                nc.vector.tensor_copy(
                                    o[:, j * SB:(j + 1) * SB], po)
                        nc.sync.dma_start(
                            out=part_d[st * P:(st + 1) * P, :], in_=o)
                    return t

                # ---------------- schedule ----------------
                # inline lead: only what h0's first steps strictly need
                # (K0/Q0 for s,t < 512-1024, V pair-0 tiles 0-3); the rest
                # drips as deadline-ordered fillers during h0-ssb0
                for t in (qk_subs(0, "k", (0,)) + v_subs(0, (0, 1, 2, 3))
                          + qk_subs(0, "q", (0, 1))):
                    t()
                # deadline-ordered h0-ssb0 fillers at 9 pops/step: K0-sb_i
                # EMITTED by step 4i, v0_st by step st (emission order is
                # what guarantees readers see written tiles)
                fill = (qk_subs(0, "k", (1,)) + v_subs(0, (4, 5))
                        + qk_subs(0, "k", (2,)) + v_subs(0, (6, 7, 8))
                        + qk_subs(0, "k", (3,))
                        + v_subs(0, (9, 10, 11, 12, 13, 14, 15))
                        + qk_subs(0, "q", (2, 3))
                        + v_subs(1)
                        + qk_subs(1, "k") + qk_subs(1, "q", (0, 1)))
                attention(0, fill, rate=(9.0, 3.2))
                rec_chain(0)
                attention(1, fill, rate=3.2)
                while fill:
                    fill.pop(0)()
                rec_chain(1)
                fill2 = qk_subs(1, "q", (2, 3)) + [
                    outproj_p0(st, j) for st in range(NST // 2)
                    for j in range(2)]
                attention(2, fill2, rate=1.2)
                rec_chain(2)

                def h3_mid():
                    # after h3's first superblock: normalize its s<1024 rows,
                    # then finish the first-half output projection as filler
                    while fill2:
                        fill2.pop(0)()
                    rec_chain(3, ssbs=(0,))
                    fill2.extend(outproj_p1(st) for st in range(NST // 2))

                attention(3, fill2, rate=1.0, mid=h3_mid)
                while fill2:
                    fill2.pop(0)()
                # per-half tail: outproj for s in [1024,1536) starts right
                # after the first half-reciprocal; the second half's
                # normalization overlaps it
                rec_chain(3, ssbs=(1,), halves=(0,))
                outproj(8, True)()
                rec_chain(3, ssbs=(1,), halves=(1,))
                for st in range(9, NST):
                    outproj(st, True)()

    nc.compile()
    return nc


_CACHE = {}


def _get_program(repeat=1):
    key = repeat
    if key not in _CACHE:
        _CACHE[key] = build_program(repeat)
    return _CACHE[key]


def _make_in_maps(inputs):
    X = np.asarray(inputs["X"], dtype=np.float32)
    mask = np.asarray(inputs["mask"], dtype=np.float32)
    Wq = np.asarray(inputs["Wq"], dtype=np.float32)
    Wk = np.asarray(inputs["Wk"], dtype=np.float32)
    Wv = np.asarray(inputs["Wv"], dtype=np.float32)
    Wo = np.asarray(inputs["Wo"], dtype=np.float32)
    bq = np.asarray(inputs["bq"], dtype=np.float32)
    bk = np.asarray(inputs["bk"], dtype=np.float32)
    bv = np.asarray(inputs["bv"], dtype=np.float32)

    f16 = np.float16
    in_maps = []
    xts = [np.ascontiguousarray(X[b].T).astype(f16) for b in range(B)]
    maskbs = [np.ascontiguousarray(-1e6 * (1.0 - mask[b])) for b in range(B)]
    for c in range(NCORES):
        b = c // 4
        g = c % 4
        cols = slice(g * DQ, (g + 1) * DQ)
        mb2 = (maskbs[b].reshape(NST, P).T + EXP_SHIFT).astype(np.float32)
        wo2 = Wo[cols, :].reshape(2, P, H).transpose(1, 0, 2)
        in_maps.append({
            "xt": xts[b],
            "wq": np.ascontiguousarray(Wq[:, cols]).astype(f16),
            "wk": np.ascontiguousarray(Wk[:, cols]).astype(f16),
            "wv": np.ascontiguousarray(Wv[:, cols]).astype(f16),
            "wo": np.ascontiguousarray(wo2),
            "bq": np.ascontiguousarray(bq[cols].reshape(2, P).T),
            "bk": np.ascontiguousarray(bk[cols].reshape(2, P).T),
            "bvb": np.ascontiguousarray(
                np.tile(bv[cols].reshape(1, DQ), (P, 1))).astype(np.float32),
            "maskb": np.ascontiguousarray(mb2),
        })
    return in_maps


def kernel(X, mask, Wq, bq, Wk, bk, Wv, bv, Wo, bo):
    bo = np.asarray(bo, dtype=np.float32)
    nc = _get_program()
    in_maps = _make_in_maps(dict(X=X, mask=mask, Wq=Wq, bq=bq, Wk=Wk, bk=bk,
                                 Wv=Wv, bv=bv, Wo=Wo, bo=bo))
    res = run_bass_kernel_spmd(nc, in_maps, list(range(NCORES))).results
    out = np.zeros((B, S, H), dtype=np.float32)
    for c in range(NCORES):
        out[c // 4] += res[c]["part"]
    out += bo
    return out


# revision 90
# speedup vs baseline: 1.0660x; 1.0083x over previous
"""Multi-head attention (B=2, S=2048, H=1024, 16 heads x 64) on 8 NeuronCores.

Sharding: tensor-parallel over heads x data-parallel over batch.
Core c handles batch (c // 4) and heads [4*(c%4), 4*(c%4)+4).
Each core computes its 4 heads' QKV projections, attention, and the partial
output projection ctx_h @ Wo_h; the host sums the 4 partials per batch.

The datapath is fp16 (noise ~5e-4; fp8 was tried and its ~2.5%/stage
quantization noise transfers 1:1 through the softmax-weighted mean, far
over the accuracy budget). fp16 matmuls run at the same 1 cycle/row as
fp32r but with half the SBUF/DMA traffic. Structural savings vs the fp32
baseline:
 - V is computed directly in [t, dv] layout by making X the stationary
   matmul operand, eliminating all PE transposes and their drains.
 - The output projection packs the two heads of a pair on the contraction
   dim (K=128 instead of 64), halving its PE time. For the first half of
   the sequence it runs as two passes overlapped with late attention
   (pair 0 during h2/h3, pair 1 as h3 filler); the second half runs
   single-pass at the end with drains alternating DVE/ACT.
 - exp outputs fp16 directly (with a -4 global shift so e^score stays in
   range; the shift cancels in the softmax ratio), halving e-tile traffic.
Softmax skips max-subtraction and gets its denominator for free from an
appended ones-column on V; 1/den is broadcast over dv rows with a K=1 PE
outer product (no DRAM round trip). ctx runs 3 t-tiles behind exp so the
in-order PE never waits on ACT latency; projections drip in as
single-matmul filler sub-tasks whose emission order respects each
consumer's deadline (the tile framework only syncs in emission order).
"""
import numpy as np

import concourse.bass as bass
import concourse.tile as tile
from concourse import bacc, mybir
from concourse.bass_utils import run_bass_kernel_spmd

F32 = mybir.dt.float32
F32R = mybir.dt.float32r
F16 = mybir.dt.float16

H, NH, HD = 1024, 16, 64
B, S = 2, 2048
P = 128
NCORES = 8
NHL = 4          # heads per core
DQ = NHL * HD    # 256 projection cols per core
NHT = H // P     # 8 h-tiles
NST = S // P     # 16 t-tiles (also s-tiles)
SB = 512         # matmul free-dim block
SS = 1024        # attention s-superblock (2 PSUM banks)
NSB = S // SB    # 4
NSS = S // SS    # 2

EXP_SHIFT = -4.0  # global exp shift (cancels in softmax); keeps e^score
                  # well inside fp16 range for scores up to ~14


def _reshape_free(ap, dims):
    """Reinterpret a contiguous free region of `ap` as `dims`."""
    total = 1
    new = []
    for d in reversed(dims):
        new.append([total, d])
        total *= d
    assert total == ap.free_size()
    return bass.AP(tensor=ap.tensor, offset=ap.offset,
                   ap=[ap.ap[0]] + list(reversed(new)))


def build_program(repeat=1):
    nc = bacc.Bacc("TRN2", target_bir_lowering=False, debug=False,
                   num_devices=NCORES)
    _lp = nc.allow_low_precision(reason="fp16 attention pipeline")
    _lp.__enter__()

    xt_d = nc.dram_tensor("xt", [H, S], F16, kind="ExternalInput").ap()
    wq_d = nc.dram_tensor("wq", [H, DQ], F16, kind="ExternalInput").ap()
    wk_d = nc.dram_tensor("wk", [H, DQ], F16, kind="ExternalInput").ap()
    wv_d = nc.dram_tensor("wv", [H, DQ], F16, kind="ExternalInput").ap()
    wo_d = nc.dram_tensor("wo", [P, 2, H], F32R, kind="ExternalInput").ap()
    bq_d = nc.dram_tensor("bq", [P, 2], F32, kind="ExternalInput").ap()
    bk_d = nc.dram_tensor("bk", [P, 2], F32, kind="ExternalInput").ap()
    bvb_d = nc.dram_tensor("bvb", [P, DQ], F32, kind="ExternalInput").ap()
    mb_d = nc.dram_tensor("maskb", [P, NST], F32, kind="ExternalInput").ap()
    part_d = nc.dram_tensor("part", [S, H], F16, kind="ExternalOutput").ap()

    with tile.TileContext(nc) as tc:
        with tc.tile_pool(name="big", bufs=1) as big, \
             tc.tile_pool(name="consts", bufs=1) as consts, \
             tc.tile_pool(name="epool", bufs=5) as epool, \
             tc.tile_pool(name="bcpool", bufs=2) as bcpool, \
             tc.tile_pool(name="opool", bufs=1) as opool, \
             tc.tile_pool(name="dpool", bufs=2) as dpool, \
             tc.tile_pool(name="ps_sc", bufs=2, space="PSUM") as ps_sc, \
             tc.tile_pool(name="ps_ctx", bufs=1, space="PSUM") as ps_ctx, \
             tc.tile_pool(name="ps_mm", bufs=2, space="PSUM") as ps_mm:

            for _it in range(repeat):
                # ---------------- input loads ----------------
                xt_sb = big.tile([P, NHT, S], F16, tag="xt", name="xt_sb")
                xt_r = xt_d.rearrange("(n p) s -> n p s", p=P)
                wq_sb = consts.tile([P, NHT, DQ], F16, tag="wq", name="wq_sb")
                wk_sb = consts.tile([P, NHT, DQ], F16, tag="wk", name="wk_sb")
                wv_sb = consts.tile([P, NHT, DQ], F16, tag="wv", name="wv_sb")

                xt_rp = xt_d.rearrange("(n p) s -> p n s", p=P)

                def load_x_cols(c0, c1):
                    nc.sync.dma_start(
                        out=xt_sb[:, :, c0:c1], in_=xt_rp[:, :, c0:c1])

                def load_w(w_sb, w_d):
                    nc.sync.dma_start(
                        out=w_sb, in_=w_d.rearrange("(n p) d -> p n d", p=P))

                load_w(wk_sb, wk_d)
                load_x_cols(0, 256)
                load_x_cols(256, 512)
                load_w(wv_sb, wv_d)
                load_w(wq_sb, wq_d)
                load_x_cols(512, 1024)
                load_x_cols(1024, 1536)
                load_x_cols(1536, 2048)

                # tiny tensors ride the idle gpsimd queue so they land in
                # the first few us instead of behind the X stream (the
                # first exp needs mb, the first drains need bq/bk/bvb)
                bq_sb = consts.tile([P, 2], F32, tag="bq", name="bq_sb")
                bk_sb = consts.tile([P, 2], F32, tag="bk", name="bk_sb")
                nc.gpsimd.dma_start(out=bq_sb, in_=bq_d)
                nc.gpsimd.dma_start(out=bk_sb, in_=bk_d)
                mb_sb = consts.tile([P, NST], F32, tag="mb", name="mb_sb")
                nc.gpsimd.dma_start(out=mb_sb, in_=mb_d)
                bvb_sb = consts.tile([P, DQ], F32, tag="bvb", name="bvb_sb")
                nc.gpsimd.dma_start(out=bvb_sb, in_=bvb_d)
                wo_sb = consts.tile([P, 2, H], F32R, tag="wo", name="wo_sb")
                nc.sync.dma_start(out=wo_sb, in_=wo_d)

                # projection outputs: Q^T/K^T in [dv(2 heads), pair, s]
                qT = big.tile([P, 2, S], F16, tag="qT", name="qT")
                kT = big.tile([P, 2, S], F16, tag="kT", name="kT")
                # V (+ones col) in [t, st, head, dv] layout
                vaug = big.tile([P, NST, NHL, HD + 1], F16, tag="vaug",
                                name="vaug")
                nc.vector.memset(vaug[:, :, :, HD:HD + 1], 1.0)

                ctx2 = [big.tile([P, S], F32R, tag=f"ctx2_{pr}",
                                 name=f"ctx2_{pr}") for pr in range(2)]

                rec_rows = {}
                ones128 = consts.tile([1, P], F32R, tag="ones128",
                                      name="ones128")
                one = nc.const_aps.aps[(F32, 1.0)]
                ones_src = bass.AP(tensor=one.tensor, offset=one.offset,
                                   ap=[[one.ap[0][0], 1], [0, P]])
                nc.vector.tensor_copy(ones128, ones_src)

                # dummy exp to pull the ACT Exp-table load (1.3us) into the
                # DMA-bound lead instead of the first real exp's critical path
                warm = consts.tile([1, 1], F16, tag="warm", name="warm")
                nc.scalar.activation(out=warm, in_=ones128[0:1, 0:1],
                                     func=mybir.ActivationFunctionType.Exp,
                                     bias=0.0, scale=1.0)

                # ---------------- projection tasks ----------------
                # emitted as single-matmul sub-tasks (~0.2us each) so filler
                # pops never stall the exp-paced attention pipeline
                def qk_subs(dqt, projs="qk", sbs=tuple(range(NSB))):
                    sel = {"q": (wq_sb, bq_sb, qT, "q"),
                           "k": (wk_sb, bk_sb, kT, "k")}
                    subs = []
                    for sb_i in sbs:
                        for w_sb, b_sb, out_sb, nm in (sel[p] for p in projs):
                            st8 = {}

                            def mm(ht, w_sb=w_sb, sb_i=sb_i, st8=st8, nm=nm):
                                def t():
                                    if ht == 0:
                                        st8["acc"] = ps_mm.tile(
                                            [P, SB], F32, tag="mm512",
                                            name=f"acc_{nm}{dqt}_{sb_i}")
                                    nc.tensor.matmul(
                                        st8["acc"],
                                        w_sb[:, ht, dqt * P:(dqt + 1) * P],
                                        xt_sb[:, ht,
                                              sb_i * SB:(sb_i + 1) * SB],
                                        start=(ht == 0), stop=(ht == NHT - 1))
                                return t

                            def drain(b_sb=b_sb, out_sb=out_sb, sb_i=sb_i,
                                      st8=st8):
                                def t():
                                    nc.vector.tensor_scalar_add(
                                        out_sb[:, dqt,
                                               sb_i * SB:(sb_i + 1) * SB],
                                        st8["acc"], b_sb[:, dqt:dqt + 1])
                                return t

                            subs += [mm(ht) for ht in range(NHT)]
                            subs.append(drain())
                    return subs

                def v_subs(dqt, sts=tuple(range(NST))):
                    subs = []
                    for st in sts:
                        st8 = {}

                        def mm(ht, st=st, st8=st8):
                            def t():
                                if ht == 0:
                                    st8["acc"] = ps_mm.tile(
                                        [P, SB], F32, tag="mm512",
                                        name=f"vacc{dqt}_{st}")
                                nc.tensor.matmul(
                                    st8["acc"][:, 0:P],
                                    xt_sb[:, ht, st * P:(st + 1) * P],
                                    wv_sb[:, ht, dqt * P:(dqt + 1) * P],
                                    start=(ht == 0), stop=(ht == NHT - 1))
                            return t

                        def drain(st=st, st8=st8):
                            def t():
                                nc.vector.tensor_add(
                                    vaug[:, st, 2 * dqt:2 * dqt + 2, 0:HD],
                                    _reshape_free(st8["acc"][:, 0:P], [2, HD]),
                                    _reshape_free(
                                        bvb_sb[:, dqt * P:(dqt + 1) * P],
                                        [2, HD]))
                            return t

                        subs += [mm(ht) for ht in range(NHT)]
                        subs.append(drain())
                    return subs

                # ---------------- attention ----------------
                def attention(h, filler, rate=2.0, mid=None):
                    base = HD * (h % 2)
                    dvt = h // 2
                    pr = h // 2
                    row = HD * (h % 2)
                    budget = 0.0
                    rates = rate if isinstance(rate, tuple) else (rate, rate)
                    for ssb in range(NSS):
                        rate = rates[ssb]
                        if ssb == 1 and mid is not None:
                            mid()
                        acc = ps_ctx.tile([HD + 1, SS], F32, tag="ctxps",
                                          name=f"ctx_{h}_{ssb}")
                        es = {}
                        # ctx runs TWO t-tiles behind exp so the PE (in-order)
                        # never waits on the ACT exp latency or its semaphore
                        LAG = 3
                        for tt in range(NST + LAG):
                            budget += rate
                            while filler and budget >= 1.0:
                                filler.pop(0)()
                                budget -= 1.0
                            if tt < NST:
                                sc = ps_sc.tile([P, SS], F32, tag="sc",
                                                name=f"sc_{h}_{ssb}_{tt}")
                                for half in range(2):
                                    sb_i = 2 * ssb + half
                                    nc.tensor.matmul(
                                        sc[:, half * SB:(half + 1) * SB],
                                        kT[base:base + HD, dvt,
                                           tt * P:(tt + 1) * P],
                                        qT[base:base + HD, dvt,
                                           sb_i * SB:(sb_i + 1) * SB],
                                        start=True, stop=True)
                            if tt >= LAG:
                                e_in = es.pop(tt - LAG)
                                for half in range(2):
                                    nc.tensor.matmul(
                                        acc[:, half * SB:(half + 1) * SB],
                                        vaug[:, tt - LAG, h, :],
                                        e_in[:, half * SB:(half + 1) * SB],
                                        start=(tt == LAG),
                                        stop=(tt == NST + LAG - 1))
                            if tt < NST:
                                e = epool.tile([P, SS], F16, tag="e",
                                               name=f"e_{h}_{ssb}_{tt}")
                                nc.scalar.activation(
                                    out=e, in_=sc,
                                    func=mybir.ActivationFunctionType.Exp,
                                    bias=mb_sb[:, tt:tt + 1], scale=1.0 / 8.0)
                                es[tt] = e
                        # denominator reciprocals FIRST (the PE's broadcast
                        # matmul waits only on these, not the drains), then
                        # drain ctx; all on-chip. The kernel's last
                        # superblock keeps per-half recips so the tail can
                        # start after half a row.
                        rec_row = dpool.tile([1, SS], F32R, tag="recrow",
                                             name=f"recrow_{h}_{ssb}")
                        if h == NHL - 1 and ssb == NSS - 1:
                            nc.vector.reciprocal(rec_row[:, 0:SB],
                                                 acc[HD:HD + 1, 0:SB])
                            nc.vector.reciprocal(rec_row[:, SB:SS],
                                                 acc[HD:HD + 1, SB:SS])
                        else:
                            nc.vector.reciprocal(rec_row, acc[HD:HD + 1, :])
                        rec_rows[(h, ssb)] = rec_row
                        nc.vector.tensor_copy(
                            ctx2[pr][row:row + HD,
                                     ssb * SS:(ssb + 1) * SS],
                            acc[0:HD, :])

                def rec_chain(h, ssbs=(0, 1), halves=(0, 1)):
                    # broadcast 1/den over the dv rows with a K=1 PE outer
                    # product (ones128 x rec_row) and scale ctx2 in place --
                    # fully on-chip, no DRAM round trip
                    pr = h // 2
                    row = HD * (h % 2)
                    for ssb in ssbs:
                        rr = rec_rows[(h, ssb)]
                        for half in halves:
                            sb_i = 2 * ssb + half
                            bc = ps_mm.tile([P, SB], F32, tag="mm512",
                                            name=f"bc_{h}_{sb_i}")
                            nc.tensor.matmul(
                                bc, ones128,
                                rr[:, half * SB:(half + 1) * SB],
                                start=True, stop=True)
                            nc.vector.tensor_mul(
                                ctx2[pr][row:row + HD,
                                         sb_i * SB:(sb_i + 1) * SB],
                                ctx2[pr][row:row + HD,
                                         sb_i * SB:(sb_i + 1) * SB],
                                bc[row:row + HD, :])

                # ---------------- output projection ----------------
                o_st = [None] * NST

                def outproj_p0(st, j):
                    def t():
                        if j == 0:
                            o_st[st] = opool.tile([P, H], F16, tag=f"o_{st}",
                                                  name=f"o_{st}")
                        o = o_st[st]
                        po = ps_mm.tile([P, SB], F32, tag="mm512",
                                        name=f"po0_{st}_{j}")
                        nc.tensor.matmul(
                            po,
                            ctx2[0][:, st * P:(st + 1) * P],
                            wo_sb[:, 0, j * SB:(j + 1) * SB],
                            start=True, stop=True)
                        nc.vector.tensor_copy(o[:, j * SB:(j + 1) * SB], po)
                    return t

                def outproj_p1(st):
                    def t():
                        o = o_st[st]
                        for j in range(2):
                            po = ps_mm.tile([P, SB], F32, tag="mm512",
                                            name=f"po1_{st}_{j}")
                            nc.tensor.matmul(
                                po,
                                ctx2[1][:, st * P:(st + 1) * P],
                                wo_sb[:, 1, j * SB:(j + 1) * SB],
                                start=True, stop=True)
                            nc.vector.tensor_add(
                                o[:, j * SB:(j + 1) * SB],
                                po, o[:, j * SB:(j + 1) * SB])
                        nc.sync.dma_start(
                            out=part_d[st * P:(st + 1) * P, :], in_=o)
                    return t

                def outproj(st, use_act):
                    # single pass over both head pairs; at the kernel tail
                    # the drains alternate DVE / ACT so neither paces it,
                    # and po tiles alternate ps_mm / the (now idle) score
                    # pool so PSUM rotation latency doesn't pace it either
                    def t():
                        o = opool.tile([P, H], F16, tag=f"o_{st}",
                                       name=f"o_{st}")
                        for j in range(2):
                            if use_act and j % 2 == 1:
                                po = ps_sc.tile([P, SS], F32, tag="sc",
                                                name=f"po_{st}_{j}")[:, 0:SB]
                            else:
                                po = ps_mm.tile([P, SB], F32, tag="mm512",
                                                name=f"po_{st}_{j}")
                            for pr in range(2):
                                nc.tensor.matmul(
                                    po,
                                    ctx2[pr][:, st * P:(st + 1) * P],
                                    wo_sb[:, pr, j * SB:(j + 1) * SB],
                                    start=(pr == 0), stop=(pr == 1))
                            if use_act and j % 2 == 1:
                                nc.scalar.copy(o[:, j * SB:(j + 1) * SB], po)
                            else:
                                nc.vector.tensor_copy(
                                    o[:, j * SB:(j + 1) * SB], po)
                        nc.sync.dma_start(
                            out=part_d[st * P:(st + 1) * P, :], in_=o)
                    return t

                # ---------------- schedule ----------------
                # inline lead: only what h0's first steps strictly need
                # (K0/Q0 for s,t < 512-1024, V pair-0 tiles 0-3); the rest
                # drips as deadline-ordered fillers during h0-ssb0
                for t in (qk_subs(0, "k", (0,)) + v_subs(0, (0, 1, 2, 3))
                          + qk_subs(0, "q", (0, 1))):
                    t()
                # deadline-ordered h0-ssb0 fillers at 9 pops/step: K0-sb_i
                # EMITTED by step 4i, v0_st by step st (emission order is
                # what guarantees readers see written tiles)
                fill = (qk_subs(0, "k", (1,)) + v_subs(0, (4, 5))
                        + qk_subs(0, "k", (2,)) + v_subs(0, (6, 7, 8))
                        + qk_subs(0, "k", (3,))
                        + v_subs(0, (9, 10, 11, 12, 13, 14, 15))
                        + qk_subs(0, "q", (2, 3))
                        + v_subs(1)
                        + qk_subs(1, "k") + qk_subs(1, "q", (0, 1)))
                attention(0, fill, rate=(9.0, 3.2))
                rec_chain(0)
                attention(1, fill, rate=3.2)
                while fill:
                    fill.pop(0)()
                rec_chain(1)
                fill2 = qk_subs(1, "q", (2, 3)) + [
                    outproj_p0(st, j) for st in range(NST // 2)
                    for j in range(2)]
                attention(2, fill2, rate=1.2)
                rec_chain(2)

                def h3_mid():
                    # after h3's first superblock: normalize its s<1024 rows,
                    # then finish the first-half output projection as filler
                    while fill2:
                        fill2.pop(0)()
                    rec_chain(3, ssbs=(0,))
                    fill2.extend(outproj_p1(st) for st in range(NST // 2))

                attention(3, fill2, rate=1.0, mid=h3_mid)
                while fill2:
                    fill2.pop(0)()
                # per-half tail: outproj for s in [1024,1536) starts right
                # after the first half-reciprocal; the second half's
                # normalization overlaps it
                rec_chain(3, ssbs=(1,), halves=(0,))
                outproj(8, True)()
                rec_chain(3, ssbs=(1,), halves=(1,))
                for st in range(9, NST):
                    outproj(st, True)()

    nc.compile()
    return nc


_CACHE = {}


def _get_program(repeat=1):
    key = repeat
    if key not in _CACHE:
        _CACHE[key] = build_program(repeat)
    return _CACHE[key]


def _make_in_maps(inputs):
    X = np.asarray(inputs["X"], dtype=np.float32)
    mask = np.asarray(inputs["mask"], dtype=np.float32)
    Wq = np.asarray(inputs["Wq"], dtype=np.float32)
    Wk = np.asarray(inputs["Wk"], dtype=np.float32)
    Wv = np.asarray(inputs["Wv"], dtype=np.float32)
    Wo = np.asarray(inputs["Wo"], dtype=np.float32)
    bq = np.asarray(inputs["bq"], dtype=np.float32)
    bk = np.asarray(inputs["bk"], dtype=np.float32)
    bv = np.asarray(inputs["bv"], dtype=np.float32)

    f16 = np.float16
    in_maps = []
    xts = [np.ascontiguousarray(X[b].T).astype(f16) for b in range(B)]
    maskbs = [np.ascontiguousarray(-1e6 * (1.0 - mask[b])) for b in range(B)]
    for c in range(NCORES):
        b = c // 4
        g = c % 4
        cols = slice(g * DQ, (g + 1) * DQ)
        mb2 = (maskbs[b].reshape(NST, P).T + EXP_SHIFT).astype(np.float32)
        wo2 = Wo[cols, :].reshape(2, P, H).transpose(1, 0, 2)
        in_maps.append({
            "xt": xts[b],
            "wq": np.ascontiguousarray(Wq[:, cols]).astype(f16),
            "wk": np.ascontiguousarray(Wk[:, cols]).astype(f16),
            "wv": np.ascontiguousarray(Wv[:, cols]).astype(f16),
            "wo": np.ascontiguousarray(wo2),
            "bq": np.ascontiguousarray(bq[cols].reshape(2, P).T),
            "bk": np.ascontiguousarray(bk[cols].reshape(2, P).T),
            "bvb": np.ascontiguousarray(
                np.tile(bv[cols].reshape(1, DQ), (P, 1))).astype(np.float32),
            "maskb": np.ascontiguousarray(mb2),
        })
    return in_maps


def kernel(X, mask, Wq, bq, Wk, bk, Wv, bv, Wo, bo):
    bo = np.asarray(bo, dtype=np.float32)
    nc = _get_program()
    in_maps = _make_in_maps(dict(X=X, mask=mask, Wq=Wq, bq=bq, Wk=Wk, bk=bk,
                                 Wv=Wv, bv=bv, Wo=Wo, bo=bo))
    res = run_bass_kernel_spmd(nc, in_maps, list(range(NCORES))).results
    out = np.zeros((B, S, H), dtype=np.float32)
    for c in range(NCORES):
        out[c // 4] += res[c]["part"]
    out += bo
    return out


# revision 92
# speedup vs baseline: 1.0663x; 1.0003x over previous
"""Multi-head attention (B=2, S=2048, H=1024, 16 heads x 64) on 8 NeuronCores.

Sharding: tensor-parallel over heads x data-parallel over batch.
Core c handles batch (c // 4) and heads [4*(c%4), 4*(c%4)+4).
Each core computes its 4 heads' QKV projections, attention, and the partial
output projection ctx_h @ Wo_h; the host sums the 4 partials per batch.

The datapath is fp16 (noise ~5e-4; fp8 was tried and its ~2.5%/stage
quantization noise transfers 1:1 through the softmax-weighted mean, far
over the accuracy budget). fp16 matmuls run at the same 1 cycle/row as
fp32r but with half the SBUF/DMA traffic. Structural savings vs the fp32
baseline:
 - V is computed directly in [t, dv] layout by making X the stationary
   matmul operand, eliminating all PE transposes and their drains.
 - The output projection packs the two heads of a pair on the contraction
   dim (K=128 instead of 64), halving its PE time. For the first half of
   the sequence it runs as two passes overlapped with late attention
   (pair 0 during h2/h3, pair 1 as h3 filler); the second half runs
   single-pass at the end with drains alternating DVE/ACT.
 - exp outputs fp16 directly (with a -4 global shift so e^score stays in
   range; the shift cancels in the softmax ratio), halving e-tile traffic.
Softmax skips max-subtraction and gets its denominator for free from an
appended ones-column on V; 1/den is broadcast over dv rows with a K=1 PE
outer product (no DRAM round trip). ctx runs 3 t-tiles behind exp so the
in-order PE never waits on ACT latency; projections drip in as
single-matmul filler sub-tasks whose emission order respects each
consumer's deadline (the tile framework only syncs in emission order).
"""
import numpy as np

import concourse.bass as bass
import concourse.tile as tile
from concourse import bacc, mybir
from concourse.bass_utils import run_bass_kernel_spmd

F32 = mybir.dt.float32
F32R = mybir.dt.float32r
F16 = mybir.dt.float16

H, NH, HD = 1024, 16, 64
B, S = 2, 2048
P = 128
NCORES = 8
NHL = 4          # heads per core
DQ = NHL * HD    # 256 projection cols per core
NHT = H // P     # 8 h-tiles
NST = S // P     # 16 t-tiles (also s-tiles)
SB = 512         # matmul free-dim block
SS = 1024        # attention s-superblock (2 PSUM banks)
NSB = S // SB    # 4
NSS = S // SS    # 2

EXP_SHIFT = -4.0  # global exp shift (cancels in softmax); keeps e^score
                  # well inside fp16 range for scores up to ~14


def _reshape_free(ap, dims):
    """Reinterpret a contiguous free region of `ap` as `dims`."""
    total = 1
    new = []
    for d in reversed(dims):
        new.append([total, d])
        total *= d
    assert total == ap.free_size()
    return bass.AP(tensor=ap.tensor, offset=ap.offset,
                   ap=[ap.ap[0]] + list(reversed(new)))


def build_program(repeat=1):
    nc = bacc.Bacc("TRN2", target_bir_lowering=False, debug=False,
                   num_devices=NCORES)
    _lp = nc.allow_low_precision(reason="fp16 attention pipeline")
    _lp.__enter__()

    xt_d = nc.dram_tensor("xt", [H, S], F16, kind="ExternalInput").ap()
    wq_d = nc.dram_tensor("wq", [H, DQ], F16, kind="ExternalInput").ap()
    wk_d = nc.dram_tensor("wk", [H, DQ], F16, kind="ExternalInput").ap()
    wv_d = nc.dram_tensor("wv", [H, DQ], F16, kind="ExternalInput").ap()
    wo_d = nc.dram_tensor("wo", [P, 2, H], F32R, kind="ExternalInput").ap()
    bq_d = nc.dram_tensor("bq", [P, 2], F32, kind="ExternalInput").ap()
    bk_d = nc.dram_tensor("bk", [P, 2], F32, kind="ExternalInput").ap()
    bvb_d = nc.dram_tensor("bvb", [P, DQ], F32, kind="ExternalInput").ap()
    mb_d = nc.dram_tensor("maskb", [P, NST], F32, kind="ExternalInput").ap()
    part_d = nc.dram_tensor("part", [S, H], F16, kind="ExternalOutput").ap()

    with tile.TileContext(nc) as tc:
        with tc.tile_pool(name="big", bufs=1) as big, \
             tc.tile_pool(name="consts", bufs=1) as consts, \
             tc.tile_pool(name="epool", bufs=5) as epool, \
             tc.tile_pool(name="bcpool", bufs=2) as bcpool, \
             tc.tile_pool(name="opool", bufs=1) as opool, \
             tc.tile_pool(name="dpool", bufs=2) as dpool, \
             tc.tile_pool(name="ps_sc", bufs=2, space="PSUM") as ps_sc, \
             tc.tile_pool(name="ps_ctx", bufs=1, space="PSUM") as ps_ctx, \
             tc.tile_pool(name="ps_mm", bufs=2, space="PSUM") as ps_mm:

            for _it in range(repeat):
                # ---------------- input loads ----------------
                xt_sb = big.tile([P, NHT, S], F16, tag="xt", name="xt_sb")
                xt_r = xt_d.rearrange("(n p) s -> n p s", p=P)
                wq_sb = consts.tile([P, NHT, DQ], F16, tag="wq", name="wq_sb")
                wk_sb = consts.tile([P, NHT, DQ], F16, tag="wk", name="wk_sb")
                wv_sb = consts.tile([P, NHT, DQ], F16, tag="wv", name="wv_sb")

                xt_rp = xt_d.rearrange("(n p) s -> p n s", p=P)

                def load_x_cols(c0, c1):
                    nc.sync.dma_start(
                        out=xt_sb[:, :, c0:c1], in_=xt_rp[:, :, c0:c1])

                def load_w(w_sb, w_d):
                    nc.sync.dma_start(
                        out=w_sb, in_=w_d.rearrange("(n p) d -> p n d", p=P))

                load_w(wk_sb, wk_d)
                load_x_cols(0, 256)
                load_x_cols(256, 512)
                load_w(wv_sb, wv_d)
                load_w(wq_sb, wq_d)
                load_x_cols(512, 1024)
                load_x_cols(1024, 1536)
                load_x_cols(1536, 2048)

                # tiny tensors ride the idle gpsimd queue so they land in
                # the first few us instead of behind the X stream (the
                # first exp needs mb, the first drains need bq/bk/bvb)
                bq_sb = consts.tile([P, 2], F32, tag="bq", name="bq_sb")
                bk_sb = consts.tile([P, 2], F32, tag="bk", name="bk_sb")
                nc.gpsimd.dma_start(out=bq_sb, in_=bq_d)
                nc.gpsimd.dma_start(out=bk_sb, in_=bk_d)
                mb_sb = consts.tile([P, NST], F32, tag="mb", name="mb_sb")
                nc.gpsimd.dma_start(out=mb_sb, in_=mb_d)
                bvb_sb = consts.tile([P, DQ], F32, tag="bvb", name="bvb_sb")
                nc.gpsimd.dma_start(out=bvb_sb, in_=bvb_d)
                wo_sb = consts.tile([P, 2, H], F32R, tag="wo", name="wo_sb")
                nc.sync.dma_start(out=wo_sb, in_=wo_d)

                # projection outputs: Q^T/K^T in [dv(2 heads), pair, s]
                qT = big.tile([P, 2, S], F16, tag="qT", name="qT")
                kT = big.tile([P, 2, S], F16, tag="kT", name="kT")
                # V (+ones col) in [t, st, head, dv] layout
                vaug = big.tile([P, NST, NHL, HD + 1], F16, tag="vaug",
                                name="vaug")
                nc.vector.memset(vaug[:, :, :, HD:HD + 1], 1.0)

                ctx2 = [big.tile([P, S], F32R, tag=f"ctx2_{pr}",
                                 name=f"ctx2_{pr}") for pr in range(2)]

                rec_rows = {}
                ones128 = consts.tile([1, P], F32R, tag="ones128",
                                      name="ones128")
                one = nc.const_aps.aps[(F32, 1.0)]
                ones_src = bass.AP(tensor=one.tensor, offset=one.offset,
                                   ap=[[one.ap[0][0], 1], [0, P]])
                nc.vector.tensor_copy(ones128, ones_src)

                # dummy exp to pull the ACT Exp-table load (1.3us) into the
                # DMA-bound lead instead of the first real exp's critical path
                warm = consts.tile([1, 1], F16, tag="warm", name="warm")
                nc.scalar.activation(out=warm, in_=ones128[0:1, 0:1],
                                     func=mybir.ActivationFunctionType.Exp,
                                     bias=0.0, scale=1.0)

                # ---------------- projection tasks ----------------
                # emitted as single-matmul sub-tasks (~0.2us each) so filler
                # pops never stall the exp-paced attention pipeline
                def qk_subs(dqt, projs="qk", sbs=tuple(range(NSB))):
                    sel = {"q": (wq_sb, bq_sb, qT, "q"),
                           "k": (wk_sb, bk_sb, kT, "k")}
                    subs = []
                    for sb_i in sbs:
                        for w_sb, b_sb, out_sb, nm in (sel[p] for p in projs):
                            st8 = {}

                            def mm(ht, w_sb=w_sb, sb_i=sb_i, st8=st8, nm=nm):
                                def t():
                                    if ht == 0:
                                        st8["acc"] = ps_mm.tile(
                                            [P, SB], F32, tag="mm512",
                                            name=f"acc_{nm}{dqt}_{sb_i}")
                                    nc.tensor.matmul(
                                        st8["acc"],
                                        w_sb[:, ht, dqt * P:(dqt + 1) * P],
                                        xt_sb[:, ht,
                                              sb_i * SB:(sb_i + 1) * SB],
                                        start=(ht == 0), stop=(ht == NHT - 1))
                                return t

                            def drain(b_sb=b_sb, out_sb=out_sb, sb_i=sb_i,
                                      st8=st8):
                                def t():
                                    nc.vector.tensor_scalar_add(
                                        out_sb[:, dqt,
                                               sb_i * SB:(sb_i + 1) * SB],
                                        st8["acc"], b_sb[:, dqt:dqt + 1])
                                return t

                            subs += [mm(ht) for ht in range(NHT)]
                            subs.append(drain())
                    return subs

                def v_subs(dqt, sts=tuple(range(NST))):
                    subs = []
                    for st in sts:
                        st8 = {}

                        def mm(ht, st=st, st8=st8):
                            def t():
                                if ht == 0:
                                    st8["acc"] = ps_mm.tile(
                                        [P, SB], F32, tag="mm512",
                                        name=f"vacc{dqt}_{st}")
                                nc.tensor.matmul(
                                    st8["acc"][:, 0:P],
                                    xt_sb[:, ht, st * P:(st + 1) * P],
                                    wv_sb[:, ht, dqt * P:(dqt + 1) * P],
                                    start=(ht == 0), stop=(ht == NHT - 1))
                            return t

                        def drain(st=st, st8=st8):
                            def t():
                                nc.vector.tensor_add(
                                    vaug[:, st, 2 * dqt:2 * dqt + 2, 0:HD],
                                    _reshape_free(st8["acc"][:, 0:P], [2, HD]),
                                    _reshape_free(
                                        bvb_sb[:, dqt * P:(dqt + 1) * P],
                                        [2, HD]))
                            return t

                        subs += [mm(ht) for ht in range(NHT)]
                        subs.append(drain())
                    return subs

                # ---------------- attention ----------------
                def attention(h, filler, rate=2.0, mid=None):
                    base = HD * (h % 2)
                    dvt = h // 2
                    pr = h // 2
                    row = HD * (h % 2)
                    budget = 0.0
                    rates = rate if isinstance(rate, tuple) else (rate, rate)
                    for ssb in range(NSS):
                        rate = rates[ssb]
                        if ssb == 1 and mid is not None:
                            mid()
                        acc = ps_ctx.tile([HD + 1, SS], F32, tag="ctxps",
                                          name=f"ctx_{h}_{ssb}")
                        es = {}
                        # ctx runs TWO t-tiles behind exp so the PE (in-order)
                        # never waits on the ACT exp latency or its semaphore
                        LAG = 3
                        for tt in range(NST + LAG):
                            budget += rate
                            while filler and budget >= 1.0:
                                filler.pop(0)()
                                budget -= 1.0
                            if tt < NST:
                                sc = ps_sc.tile([P, SS], F32, tag="sc",
                                                name=f"sc_{h}_{ssb}_{tt}")
                                for half in range(2):
                                    sb_i = 2 * ssb + half
                                    nc.tensor.matmul(
                                        sc[:, half * SB:(half + 1) * SB],
                                        kT[base:base + HD, dvt,
                                           tt * P:(tt + 1) * P],
                                        qT[base:base + HD, dvt,
                                           sb_i * SB:(sb_i + 1) * SB],
                                        start=True, stop=True)
                            if tt >= LAG:
                                e_in = es.pop(tt - LAG)
                                for half in range(2):
                                    nc.tensor.matmul(
                                        acc[:, half * SB:(half + 1) * SB],
                                        vaug[:, tt - LAG, h, :],
                                        e_in[:, half * SB:(half + 1) * SB],
                                        start=(tt == LAG),
                                        stop=(tt == NST + LAG - 1))
                            if tt < NST:
                                e = epool.tile([P, SS], F16, tag="e",
                                               name=f"e_{h}_{ssb}_{tt}")
                                nc.scalar.activation(
                                    out=e, in_=sc,
                                    func=mybir.ActivationFunctionType.Exp,
                                    bias=mb_sb[:, tt:tt + 1], scale=1.0 / 8.0)
                                es[tt] = e
                        # denominator reciprocals FIRST (the PE's broadcast
                        # matmul waits only on these, not the drains), then
                        # drain ctx; all on-chip. The kernel's last
                        # superblock keeps per-half recips so the tail can
                        # start after half a row.
                        rec_row = dpool.tile([1, SS], F32R, tag="recrow",
                                             name=f"recrow_{h}_{ssb}")
                        if h == NHL - 1 and ssb == NSS - 1:
                            nc.vector.reciprocal(rec_row[:, 0:SB],
                                                 acc[HD:HD + 1, 0:SB])
                            nc.vector.reciprocal(rec_row[:, SB:SS],
                                                 acc[HD:HD + 1, SB:SS])
                        else:
                            nc.vector.reciprocal(rec_row, acc[HD:HD + 1, :])
                        rec_rows[(h, ssb)] = rec_row
                        nc.vector.tensor_copy(
                            ctx2[pr][row:row + HD,
                                     ssb * SS:(ssb + 1) * SS],
                            acc[0:HD, :])

                def rec_thunks(h, ssbs=(0, 1), halves=(0, 1)):
                    # one thunk per (ssb, half) so the normalization can be
                    # dripped as filler into the NEXT head's attention
                    # instead of blocking its scores in the in-order PE queue
                    def mk(ssb, half):
                        def t():
                            rec_one(h, ssb, half)
                        return t
                    return [mk(ssb, half) for ssb in ssbs for half in halves]

                def rec_chain(h, ssbs=(0, 1), halves=(0, 1)):
                    for ssb in ssbs:
                        for half in halves:
                            rec_one(h, ssb, half)

                def rec_one(h, ssb, half):
                    # broadcast 1/den over the dv rows with a K=1 PE outer
                    # product (ones128 x rec_row) and scale ctx2 in place --
                    # fully on-chip, no DRAM round trip
                    pr = h // 2
                    row = HD * (h % 2)
                    if True:
                        rr = rec_rows[(h, ssb)]
                        if True:
                            sb_i = 2 * ssb + half
                            bc = ps_mm.tile([P, SB], F32, tag="mm512",
                                            name=f"bc_{h}_{sb_i}")
                            nc.tensor.matmul(
                                bc, ones128,
                                rr[:, half * SB:(half + 1) * SB],
                                start=True, stop=True)
                            nc.vector.tensor_mul(
                                ctx2[pr][row:row + HD,
                                         sb_i * SB:(sb_i + 1) * SB],
                                ctx2[pr][row:row + HD,
                                         sb_i * SB:(sb_i + 1) * SB],
                                bc[row:row + HD, :])

                # ---------------- output projection ----------------
                o_st = [None] * NST

                def outproj_p0(st, j):
                    def t():
                        if j == 0:
                            o_st[st] = opool.tile([P, H], F16, tag=f"o_{st}",
                                                  name=f"o_{st}")
                        o = o_st[st]
                        po = ps_mm.tile([P, SB], F32, tag="mm512",
                                        name=f"po0_{st}_{j}")
                        nc.tensor.matmul(
                            po,
                            ctx2[0][:, st * P:(st + 1) * P],
                            wo_sb[:, 0, j * SB:(j + 1) * SB],
                            start=True, stop=True)
                        nc.vector.tensor_copy(o[:, j * SB:(j + 1) * SB], po)
                    return t

                def outproj_p1(st):
                    def t():
                        o = o_st[st]
                        for j in range(2):
                            po = ps_mm.tile([P, SB], F32, tag="mm512",
                                            name=f"po1_{st}_{j}")
                            nc.tensor.matmul(
                                po,
                                ctx2[1][:, st * P:(st + 1) * P],
                                wo_sb[:, 1, j * SB:(j + 1) * SB],
                                start=True, stop=True)
                            nc.vector.tensor_add(
                                o[:, j * SB:(j + 1) * SB],
                                po, o[:, j * SB:(j + 1) * SB])
                        nc.sync.dma_start(
                            out=part_d[st * P:(st + 1) * P, :], in_=o)
                    return t

                def outproj(st, use_act):
                    # single pass over both head pairs; at the kernel tail
                    # the drains alternate DVE / ACT so neither paces it,
                    # and po tiles alternate ps_mm / the (now idle) score
                    # pool so PSUM rotation latency doesn't pace it either
                    def t():
                        o = opool.tile([P, H], F16, tag=f"o_{st}",
                                       name=f"o_{st}")
                        for j in range(2):
                            if use_act and j % 2 == 1:
                                po = ps_sc.tile([P, SS], F32, tag="sc",
                                                name=f"po_{st}_{j}")[:, 0:SB]
                            else:
                                po = ps_mm.tile([P, SB], F32, tag="mm512",
                                                name=f"po_{st}_{j}")
                            for pr in range(2):
                                nc.tensor.matmul(
                                    po,
                                    ctx2[pr][:, st * P:(st + 1) * P],
                                    wo_sb[:, pr, j * SB:(j + 1) * SB],
                                    start=(pr == 0), stop=(pr == 1))
                            if use_act and j % 2 == 1:
                                nc.scalar.copy(o[:, j * SB:(j + 1) * SB], po)
                            else:
                                nc.vector.tensor_copy(
                                    o[:, j * SB:(j + 1) * SB], po)
                        nc.sync.dma_start(
                            out=part_d[st * P:(st + 1) * P, :], in_=o)
                    return t

                # ---------------- schedule ----------------
                # inline lead: only what h0's first steps strictly need
                # (K0/Q0 for s,t < 512-1024, V pair-0 tiles 0-3); the rest
                # drips as deadline-ordered fillers during h0-ssb0
                for t in (qk_subs(0, "k", (0,)) + v_subs(0, (0, 1, 2, 3))
                          + qk_subs(0, "q", (0, 1))):
                    t()
                # deadline-ordered h0-ssb0 fillers at 9 pops/step: K0-sb_i
                # EMITTED by step 4i, v0_st by step st (emission order is
                # what guarantees readers see written tiles)
                fill = (qk_subs(0, "k", (1,)) + v_subs(0, (4, 5))
                        + qk_subs(0, "k", (2,)) + v_subs(0, (6, 7, 8))
                        + qk_subs(0, "k", (3,))
                        + v_subs(0, (9, 10, 11, 12, 13, 14, 15))
                        + qk_subs(0, "q", (2, 3))
                        + v_subs(1)
                        + qk_subs(1, "k") + qk_subs(1, "q", (0, 1)))
                attention(0, fill, rate=(9.0, 3.2))
                fill[0:0] = rec_thunks(0)
                attention(1, fill, rate=3.2)
                while fill:
                    fill.pop(0)()
                fill2 = (rec_thunks(1) + qk_subs(1, "q", (2, 3)) + [
                    outproj_p0(st, j) for st in range(NST // 2)
                    for j in range(2)])
                attention(2, fill2, rate=1.2)
                fill2[0:0] = rec_thunks(2)

                def h3_mid():
                    # after h3's first superblock: normalize its s<1024 rows,
                    # then finish the first-half output projection as filler
                    while fill2:
                        fill2.pop(0)()
                    rec_chain(3, ssbs=(0,))
                    fill2.extend(outproj_p1(st) for st in range(NST // 2))

                attention(3, fill2, rate=1.0, mid=h3_mid)
                while fill2:
                    fill2.pop(0)()
                # per-half tail: outproj for s in [1024,1536) starts right
                # after the first half-reciprocal; the second half's
                # normalization overlaps it
                rec_chain(3, ssbs=(1,), halves=(0,))
                outproj(8, True)()
                rec_chain(3, ssbs=(1,), halves=(1,))
                for st in range(9, NST):
                    outproj(st, True)()

    nc.compile()
    return nc


_CACHE = {}


def _get_program(repeat=1):
    key = repeat
    if key not in _CACHE:
        _CACHE[key] = build_program(repeat)
    return _CACHE[key]


def _make_in_maps(inputs):
    X = np.asarray(inputs["X"], dtype=np.float32)
    mask = np.asarray(inputs["mask"], dtype=np.float32)
    Wq = np.asarray(inputs["Wq"], dtype=np.float32)
    Wk = np.asarray(inputs["Wk"], dtype=np.float32)
    Wv = np.asarray(inputs["Wv"], dtype=np.float32)
    Wo = np.asarray(inputs["Wo"], dtype=np.float32)
    bq = np.asarray(inputs["bq"], dtype=np.float32)
    bk = np.asarray(inputs["bk"], dtype=np.float32)
    bv = np.asarray(inputs["bv"], dtype=np.float32)

    f16 = np.float16
    in_maps = []
    xts = [np.ascontiguousarray(X[b].T).astype(f16) for b in range(B)]
    maskbs = [np.ascontiguousarray(-1e6 * (1.0 - mask[b])) for b in range(B)]
    for c in range(NCORES):
        b = c // 4
        g = c % 4
        cols = slice(g * DQ, (g + 1) * DQ)
        mb2 = (maskbs[b].reshape(NST, P).T + EXP_SHIFT).astype(np.float32)
        wo2 = Wo[cols, :].reshape(2, P, H).transpose(1, 0, 2)
        in_maps.append({
            "xt": xts[b],
            "wq": np.ascontiguousarray(Wq[:, cols]).astype(f16),
            "wk": np.ascontiguousarray(Wk[:, cols]).astype(f16),
            "wv": np.ascontiguousarray(Wv[:, cols]).astype(f16),
            "wo": np.ascontiguousarray(wo2),
            "bq": np.ascontiguousarray(bq[cols].reshape(2, P).T),
            "bk": np.ascontiguousarray(bk[cols].reshape(2, P).T),
            "bvb": np.ascontiguousarray(
                np.tile(bv[cols].reshape(1, DQ), (P, 1))).astype(np.float32),
            "maskb": np.ascontiguousarray(mb2),
        })
    return in_maps


def kernel(X, mask, Wq, bq, Wk, bk, Wv, bv, Wo, bo):
    bo = np.asarray(bo, dtype=np.float32)
    nc = _get_program()
    in_maps = _make_in_maps(dict(X=X, mask=mask, Wq=Wq, bq=bq, Wk=Wk, bk=bk,
                                 Wv=Wv, bv=bv, Wo=Wo, bo=bo))
    res = run_bass_kernel_spmd(nc, in_maps, list(range(NCORES))).results
    out = np.zeros((B, S, H), dtype=np.float32)
    for c in range(NCORES):
        out[c // 4] += res[c]["part"]
    out += bo
    return out


# revision 94
# speedup vs baseline: 1.0722x; 1.0055x over previous
"""Multi-head attention (B=2, S=2048, H=1024, 16 heads x 64) on 8 NeuronCores.

Sharding: tensor-parallel over heads x data-parallel over batch.
Core c handles batch (c // 4) and heads [4*(c%4), 4*(c%4)+4).
Each core computes its 4 heads' QKV projections, attention, and the partial
output projection ctx_h @ Wo_h; the host sums the 4 partials per batch.

The datapath is fp16 (noise ~5e-4; fp8 was tried and its ~2.5%/stage
quantization noise transfers 1:1 through the softmax-weighted mean, far
over the accuracy budget). fp16 matmuls run at the same 1 cycle/row as
fp32r but with half the SBUF/DMA traffic. Structural savings vs the fp32
baseline:
 - V is computed directly in [t, dv] layout by making X the stationary
   matmul operand, eliminating all PE transposes and their drains.
 - The output projection packs the two heads of a pair on the contraction
   dim (K=128 instead of 64), halving its PE time. For the first half of
   the sequence it runs as two passes overlapped with late attention
   (pair 0 during h2/h3, pair 1 as h3 filler); the second half runs
   single-pass at the end with drains alternating DVE/ACT.
 - exp outputs fp16 directly (with a -4 global shift so e^score stays in
   range; the shift cancels in the softmax ratio), halving e-tile traffic.
Softmax skips max-subtraction and gets its denominator for free from an
appended ones-column on V; 1/den is broadcast over dv rows with a K=1 PE
outer product (no DRAM round trip). ctx runs 4 t-tiles behind exp so the
in-order PE never waits on ACT latency; projections drip in as
single-matmul filler sub-tasks whose emission order respects each
consumer's deadline (the tile framework only syncs in emission order).
"""
import numpy as np

import concourse.bass as bass
import concourse.tile as tile
from concourse import bacc, mybir
from concourse.bass_utils import run_bass_kernel_spmd

F32 = mybir.dt.float32
F32R = mybir.dt.float32r
F16 = mybir.dt.float16

H, NH, HD = 1024, 16, 64
B, S = 2, 2048
P = 128
NCORES = 8
NHL = 4          # heads per core
DQ = NHL * HD    # 256 projection cols per core
NHT = H // P     # 8 h-tiles
NST = S // P     # 16 t-tiles (also s-tiles)
SB = 512         # matmul free-dim block
SS = 1024        # attention s-superblock (2 PSUM banks)
NSB = S // SB    # 4
NSS = S // SS    # 2

EXP_SHIFT = -4.0  # global exp shift (cancels in softmax); keeps e^score
                  # well inside fp16 range for scores up to ~14


def _reshape_free(ap, dims):
    """Reinterpret a contiguous free region of `ap` as `dims`."""
    total = 1
    new = []
    for d in reversed(dims):
        new.append([total, d])
        total *= d
    assert total == ap.free_size()
    return bass.AP(tensor=ap.tensor, offset=ap.offset,
                   ap=[ap.ap[0]] + list(reversed(new)))


def build_program(repeat=1):
    nc = bacc.Bacc("TRN2", target_bir_lowering=False, debug=False,
                   num_devices=NCORES)
    _lp = nc.allow_low_precision(reason="fp16 attention pipeline")
    _lp.__enter__()

    xt_d = nc.dram_tensor("xt", [H, S], F16, kind="ExternalInput").ap()
    wq_d = nc.dram_tensor("wq", [H, DQ], F16, kind="ExternalInput").ap()
    wk_d = nc.dram_tensor("wk", [H, DQ], F16, kind="ExternalInput").ap()
    wv_d = nc.dram_tensor("wv", [H, DQ], F16, kind="ExternalInput").ap()
    wo_d = nc.dram_tensor("wo", [P, 2, H], F32R, kind="ExternalInput").ap()
    bq_d = nc.dram_tensor("bq", [P, 2], F32, kind="ExternalInput").ap()
    bk_d = nc.dram_tensor("bk", [P, 2], F32, kind="ExternalInput").ap()
    bvb_d = nc.dram_tensor("bvb", [P, DQ], F32, kind="ExternalInput").ap()
    mb_d = nc.dram_tensor("maskb", [P, NST], F32, kind="ExternalInput").ap()
    part_d = nc.dram_tensor("part", [S, H], F16, kind="ExternalOutput").ap()

    with tile.TileContext(nc) as tc:
        with tc.tile_pool(name="big", bufs=1) as big, \
             tc.tile_pool(name="consts", bufs=1) as consts, \
             tc.tile_pool(name="epool", bufs=6) as epool, \
             tc.tile_pool(name="bcpool", bufs=2) as bcpool, \
             tc.tile_pool(name="opool", bufs=1) as opool, \
             tc.tile_pool(name="dpool", bufs=2) as dpool, \
             tc.tile_pool(name="ps_sc", bufs=2, space="PSUM") as ps_sc, \
             tc.tile_pool(name="ps_ctx", bufs=1, space="PSUM") as ps_ctx, \
             tc.tile_pool(name="ps_mm", bufs=2, space="PSUM") as ps_mm:

            for _it in range(repeat):
                # ---------------- input loads ----------------
                xt_sb = big.tile([P, NHT, S], F16, tag="xt", name="xt_sb")
                xt_r = xt_d.rearrange("(n p) s -> n p s", p=P)
                wq_sb = consts.tile([P, NHT, DQ], F16, tag="wq", name="wq_sb")
                wk_sb = consts.tile([P, NHT, DQ], F16, tag="wk", name="wk_sb")
                wv_sb = consts.tile([P, NHT, DQ], F16, tag="wv", name="wv_sb")

                xt_rp = xt_d.rearrange("(n p) s -> p n s", p=P)

                def load_x_cols(c0, c1):
                    nc.sync.dma_start(
                        out=xt_sb[:, :, c0:c1], in_=xt_rp[:, :, c0:c1])

                def load_w(w_sb, w_d):
                    nc.sync.dma_start(
                        out=w_sb, in_=w_d.rearrange("(n p) d -> p n d", p=P))

                load_w(wk_sb, wk_d)
                load_x_cols(0, 256)
                load_x_cols(256, 512)
                load_w(wv_sb, wv_d)
                load_w(wq_sb, wq_d)
                load_x_cols(512, 1024)
                load_x_cols(1024, 1536)
                load_x_cols(1536, 2048)

                # tiny tensors ride the idle gpsimd queue so they land in
                # the first few us instead of behind the X stream (the
                # first exp needs mb, the first drains need bq/bk/bvb)
                bq_sb = consts.tile([P, 2], F32, tag="bq", name="bq_sb")
                bk_sb = consts.tile([P, 2], F32, tag="bk", name="bk_sb")
                nc.gpsimd.dma_start(out=bq_sb, in_=bq_d)
                nc.gpsimd.dma_start(out=bk_sb, in_=bk_d)
                mb_sb = consts.tile([P, NST], F32, tag="mb", name="mb_sb")
                nc.gpsimd.dma_start(out=mb_sb, in_=mb_d)
                bvb_sb = consts.tile([P, DQ], F32, tag="bvb", name="bvb_sb")
                nc.gpsimd.dma_start(out=bvb_sb, in_=bvb_d)
                wo_sb = consts.tile([P, 2, H], F32R, tag="wo", name="wo_sb")
                nc.sync.dma_start(out=wo_sb, in_=wo_d)

                # projection outputs: Q^T/K^T in [dv(2 heads), pair, s]
                qT = big.tile([P, 2, S], F16, tag="qT", name="qT")
                kT = big.tile([P, 2, S], F16, tag="kT", name="kT")
                # V (+ones col) in [t, st, head, dv] layout
                vaug = big.tile([P, NST, NHL, HD + 1], F16, tag="vaug",
                                name="vaug")
                nc.vector.memset(vaug[:, :, :, HD:HD + 1], 1.0)

                ctx2 = [big.tile([P, S], F32R, tag=f"ctx2_{pr}",
                                 name=f"ctx2_{pr}") for pr in range(2)]

                rec_rows = {}
                ones128 = consts.tile([1, P], F32R, tag="ones128",
                                      name="ones128")
                one = nc.const_aps.aps[(F32, 1.0)]
                ones_src = bass.AP(tensor=one.tensor, offset=one.offset,
                                   ap=[[one.ap[0][0], 1], [0, P]])
                nc.vector.tensor_copy(ones128, ones_src)

                # dummy exp to pull the ACT Exp-table load (1.3us) into the
                # DMA-bound lead instead of the first real exp's critical path
                warm = consts.tile([1, 1], F16, tag="warm", name="warm")
                nc.scalar.activation(out=warm, in_=ones128[0:1, 0:1],
                                     func=mybir.ActivationFunctionType.Exp,
                                     bias=0.0, scale=1.0)

                # ---------------- projection tasks ----------------
                # emitted as single-matmul sub-tasks (~0.2us each) so filler
                # pops never stall the exp-paced attention pipeline
                def qk_subs(dqt, projs="qk", sbs=tuple(range(NSB))):
                    sel = {"q": (wq_sb, bq_sb, qT, "q"),
                           "k": (wk_sb, bk_sb, kT, "k")}
                    subs = []
                    for sb_i in sbs:
                        for w_sb, b_sb, out_sb, nm in (sel[p] for p in projs):
                            st8 = {}

                            def mm(ht, w_sb=w_sb, sb_i=sb_i, st8=st8, nm=nm):
                                def t():
                                    if ht == 0:
                                        st8["acc"] = ps_mm.tile(
                                            [P, SB], F32, tag="mm512",
                                            name=f"acc_{nm}{dqt}_{sb_i}")
                                    nc.tensor.matmul(
                                        st8["acc"],
                                        w_sb[:, ht, dqt * P:(dqt + 1) * P],
                                        xt_sb[:, ht,
                                              sb_i * SB:(sb_i + 1) * SB],
                                        start=(ht == 0), stop=(ht == NHT - 1))
                                return t

                            def drain(b_sb=b_sb, out_sb=out_sb, sb_i=sb_i,
                                      st8=st8):
                                def t():
                                    nc.vector.tensor_scalar_add(
                                        out_sb[:, dqt,
                                               sb_i * SB:(sb_i + 1) * SB],
                                        st8["acc"], b_sb[:, dqt:dqt + 1])
                                return t

                            subs += [mm(ht) for ht in range(NHT)]
                            subs.append(drain())
                    return subs

                def v_subs(dqt, sts=tuple(range(NST))):
                    subs = []
                    for st in sts:
                        st8 = {}

                        def mm(ht, st=st, st8=st8):
                            def t():
                                if ht == 0:
                                    st8["acc"] = ps_mm.tile(
                                        [P, SB], F32, tag="mm512",
                                        name=f"vacc{dqt}_{st}")
                                nc.tensor.matmul(
                                    st8["acc"][:, 0:P],
                                    xt_sb[:, ht, st * P:(st + 1) * P],
                                    wv_sb[:, ht, dqt * P:(dqt + 1) * P],
                                    start=(ht == 0), stop=(ht == NHT - 1))
                            return t

                        def drain(st=st, st8=st8):
                            def t():
                                nc.vector.tensor_add(
                                    vaug[:, st, 2 * dqt:2 * dqt + 2, 0:HD],
                                    _reshape_free(st8["acc"][:, 0:P], [2, HD]),
                                    _reshape_free(
                                        bvb_sb[:, dqt * P:(dqt + 1) * P],
                                        [2, HD]))
                            return t

                        subs += [mm(ht) for ht in range(NHT)]
                        subs.append(drain())
                    return subs

                # ---------------- attention ----------------
                def attention(h, filler, rate=2.0, mid=None):
                    base = HD * (h % 2)
                    dvt = h // 2
                    pr = h // 2
                    row = HD * (h % 2)
                    budget = 0.0
                    rates = rate if isinstance(rate, tuple) else (rate, rate)
                    for ssb in range(NSS):
                        rate = rates[ssb]
                        if ssb == 1 and mid is not None:
                            mid()
                        acc = ps_ctx.tile([HD + 1, SS], F32, tag="ctxps",
                                          name=f"ctx_{h}_{ssb}")
                        es = {}
                        # ctx runs TWO t-tiles behind exp so the PE (in-order)
                        # never waits on the ACT exp latency or its semaphore
                        LAG = 4
                        for tt in range(NST + LAG):
                            budget += rate
                            while filler and budget >= 1.0:
                                filler.pop(0)()
                                budget -= 1.0
                            if tt < NST:
                                sc = ps_sc.tile([P, SS], F32, tag="sc",
                                                name=f"sc_{h}_{ssb}_{tt}")
                                for half in range(2):
                                    sb_i = 2 * ssb + half
                                    nc.tensor.matmul(
                                        sc[:, half * SB:(half + 1) * SB],
                                        kT[base:base + HD, dvt,
                                           tt * P:(tt + 1) * P],
                                        qT[base:base + HD, dvt,
                                           sb_i * SB:(sb_i + 1) * SB],
                                        start=True, stop=True)
                            if tt >= LAG:
                                e_in = es.pop(tt - LAG)
                                for half in range(2):
                                    nc.tensor.matmul(
                                        acc[:, half * SB:(half + 1) * SB],
                                        vaug[:, tt - LAG, h, :],
                                        e_in[:, half * SB:(half + 1) * SB],
                                        start=(tt == LAG),
                                        stop=(tt == NST + LAG - 1))
                            if tt < NST:
                                e = epool.tile([P, SS], F16, tag="e",
                                               name=f"e_{h}_{ssb}_{tt}")
                                nc.scalar.activation(
                                    out=e, in_=sc,
                                    func=mybir.ActivationFunctionType.Exp,
                                    bias=mb_sb[:, tt:tt + 1], scale=1.0 / 8.0)
                                es[tt] = e
                        # denominator reciprocals FIRST (the PE's broadcast
                        # matmul waits only on these, not the drains), then
                        # drain ctx; all on-chip. The kernel's last
                        # superblock keeps per-half recips so the tail can
                        # start after half a row.
                        rec_row = dpool.tile([1, SS], F32R, tag="recrow",
                                             name=f"recrow_{h}_{ssb}")
                        if h == NHL - 1 and ssb == NSS - 1:
                            nc.vector.reciprocal(rec_row[:, 0:SB],
                                                 acc[HD:HD + 1, 0:SB])
                            nc.vector.reciprocal(rec_row[:, SB:SS],
                                                 acc[HD:HD + 1, SB:SS])
                        else:
                            nc.vector.reciprocal(rec_row, acc[HD:HD + 1, :])
                        rec_rows[(h, ssb)] = rec_row
                        nc.vector.tensor_copy(
                            ctx2[pr][row:row + HD,
                                     ssb * SS:(ssb + 1) * SS],
                            acc[0:HD, :])

                def rec_thunks(h, ssbs=(0, 1), halves=(0, 1)):
                    # one thunk per (ssb, half) so the normalization can be
                    # dripped as filler into the NEXT head's attention
                    # instead of blocking its scores in the in-order PE queue
                    def mk(ssb, half):
                        def t():
                            rec_one(h, ssb, half)
                        return t
                    return [mk(ssb, half) for ssb in ssbs for half in halves]

                def rec_chain(h, ssbs=(0, 1), halves=(0, 1)):
                    for ssb in ssbs:
                        for half in halves:
                            rec_one(h, ssb, half)

                def rec_one(h, ssb, half):
                    # broadcast 1/den over the dv rows with a K=1 PE outer
                    # product (ones128 x rec_row) and scale ctx2 in place --
                    # fully on-chip, no DRAM round trip
                    pr = h // 2
                    row = HD * (h % 2)
                    if True:
                        rr = rec_rows[(h, ssb)]
                        if True:
                            sb_i = 2 * ssb + half
                            bc = ps_mm.tile([P, SB], F32, tag="mm512",
                                            name=f"bc_{h}_{sb_i}")
                            nc.tensor.matmul(
                                bc, ones128,
                                rr[:, half * SB:(half + 1) * SB],
                                start=True, stop=True)
                            nc.vector.tensor_mul(
                                ctx2[pr][row:row + HD,
                                         sb_i * SB:(sb_i + 1) * SB],
                                ctx2[pr][row:row + HD,
                                         sb_i * SB:(sb_i + 1) * SB],
                                bc[row:row + HD, :])

                # ---------------- output projection ----------------
                o_st = [None] * NST

                def outproj_p0(st, j):
                    def t():
                        if j == 0:
                            o_st[st] = opool.tile([P, H], F16, tag=f"o_{st}",
                                                  name=f"o_{st}")
                        o = o_st[st]
                        po = ps_mm.tile([P, SB], F32, tag="mm512",
                                        name=f"po0_{st}_{j}")
                        nc.tensor.matmul(
                            po,
                            ctx2[0][:, st * P:(st + 1) * P],
                            wo_sb[:, 0, j * SB:(j + 1) * SB],
                            start=True, stop=True)
                        nc.vector.tensor_copy(o[:, j * SB:(j + 1) * SB], po)
                    return t

                def outproj_p1(st):
                    def t():
                        o = o_st[st]
                        for j in range(2):
                            po = ps_mm.tile([P, SB], F32, tag="mm512",
                                            name=f"po1_{st}_{j}")
                            nc.tensor.matmul(
                                po,
                                ctx2[1][:, st * P:(st + 1) * P],
                                wo_sb[:, 1, j * SB:(j + 1) * SB],
                                start=True, stop=True)
                            nc.vector.tensor_add(
                                o[:, j * SB:(j + 1) * SB],
                                po, o[:, j * SB:(j + 1) * SB])
                        nc.sync.dma_start(
                            out=part_d[st * P:(st + 1) * P, :], in_=o)
                    return t

                def outproj(st, use_act):
                    # single pass over both head pairs; at the kernel tail
                    # the drains alternate DVE / ACT so neither paces it,
                    # and po tiles alternate ps_mm / the (now idle) score
                    # pool so PSUM rotation latency doesn't pace it either
                    def t():
                        o = opool.tile([P, H], F16, tag=f"o_{st}",
                                       name=f"o_{st}")
                        for j in range(2):
                            if use_act and j % 2 == 1:
                                po = ps_sc.tile([P, SS], F32, tag="sc",
                                                name=f"po_{st}_{j}")[:, 0:SB]
                            else:
                                po = ps_mm.tile([P, SB], F32, tag="mm512",
                                                name=f"po_{st}_{j}")
                            for pr in range(2):
                                nc.tensor.matmul(
                                    po,
                                    ctx2[pr][:, st * P:(st + 1) * P],
                                    wo_sb[:, pr, j * SB:(j + 1) * SB],
                                    start=(pr == 0), stop=(pr == 1))
                            if use_act and j % 2 == 1:
                                nc.scalar.copy(o[:, j * SB:(j + 1) * SB], po)
                            else:
                                nc.vector.tensor_copy(
                                    o[:, j * SB:(j + 1) * SB], po)
                        nc.sync.dma_start(
                            out=part_d[st * P:(st + 1) * P, :], in_=o)
                    return t

                # ---------------- schedule ----------------
                # inline lead: only what h0's first steps strictly need
                # (K0/Q0 for s,t < 512-1024, V pair-0 tiles 0-3); the rest
                # drips as deadline-ordered fillers during h0-ssb0
                for t in (qk_subs(0, "k", (0,)) + v_subs(0, (0, 1, 2, 3))
                          + qk_subs(0, "q", (0, 1))):
                    t()
                # deadline-ordered h0-ssb0 fillers at 9 pops/step: K0-sb_i
                # EMITTED by step 4i, v0_st by step st (emission order is
                # what guarantees readers see written tiles)
                fill = (qk_subs(0, "k", (1,)) + v_subs(0, (4, 5))
                        + qk_subs(0, "k", (2,)) + v_subs(0, (6, 7, 8))
                        + qk_subs(0, "k", (3,))
                        + v_subs(0, (9, 10, 11, 12, 13, 14, 15))
                        + qk_subs(0, "q", (2, 3))
                        + v_subs(1)
                        + qk_subs(1, "k") + qk_subs(1, "q", (0, 1)))
                attention(0, fill, rate=(9.0, 3.2))
                fill[0:0] = rec_thunks(0)
                attention(1, fill, rate=3.2)
                while fill:
                    fill.pop(0)()
                fill2 = (rec_thunks(1) + qk_subs(1, "q", (2, 3)) + [
                    outproj_p0(st, j) for st in range(NST // 2)
                    for j in range(2)])
                attention(2, fill2, rate=1.2)
                fill2[0:0] = rec_thunks(2)

                def h3_mid():
                    # after h3's first superblock: normalize its s<1024 rows,
                    # then finish the first-half output projection as filler
                    while fill2:
                        fill2.pop(0)()
                    rec_chain(3, ssbs=(0,))
                    fill2.extend(outproj_p1(st) for st in range(NST // 2))

                attention(3, fill2, rate=1.0, mid=h3_mid)
                while fill2:
                    fill2.pop(0)()
                # per-half tail: outproj for s in [1024,1536) starts right
                # after the first half-reciprocal; the second half's
                # normalization overlaps it
                rec_chain(3, ssbs=(1,), halves=(0,))
                outproj(8, True)()
                rec_chain(3, ssbs=(1,), halves=(1,))
                for st in range(9, NST):
                    outproj(st, True)()

    nc.compile()
    return nc


_CACHE = {}


def _get_program(repeat=1):
    key = repeat
    if key not in _CACHE:
        _CACHE[key] = build_program(repeat)
    return _CACHE[key]


def _make_in_maps(inputs):
    X = np.asarray(inputs["X"], dtype=np.float32)
    mask = np.asarray(inputs["mask"], dtype=np.float32)
    Wq = np.asarray(inputs["Wq"], dtype=np.float32)
    Wk = np.asarray(inputs["Wk"], dtype=np.float32)
    Wv = np.asarray(inputs["Wv"], dtype=np.float32)
    Wo = np.asarray(inputs["Wo"], dtype=np.float32)
    bq = np.asarray(inputs["bq"], dtype=np.float32)
    bk = np.asarray(inputs["bk"], dtype=np.float32)
    bv = np.asarray(inputs["bv"], dtype=np.float32)

    f16 = np.float16
    in_maps = []
    xts = [np.ascontiguousarray(X[b].T).astype(f16) for b in range(B)]
    maskbs = [np.ascontiguousarray(-1e6 * (1.0 - mask[b])) for b in range(B)]
    for c in range(NCORES):
        b = c // 4
        g = c % 4
        cols = slice(g * DQ, (g + 1) * DQ)
        mb2 = (maskbs[b].reshape(NST, P).T + EXP_SHIFT).astype(np.float32)
        wo2 = Wo[cols, :].reshape(2, P, H).transpose(1, 0, 2)
        in_maps.append({
            "xt": xts[b],
            "wq": np.ascontiguousarray(Wq[:, cols]).astype(f16),
            "wk": np.ascontiguousarray(Wk[:, cols]).astype(f16),
            "wv": np.ascontiguousarray(Wv[:, cols]).astype(f16),
            "wo": np.ascontiguousarray(wo2),
            "bq": np.ascontiguousarray(bq[cols].reshape(2, P).T),
            "bk": np.ascontiguousarray(bk[cols].reshape(2, P).T),
            "bvb": np.ascontiguousarray(
                np.tile(bv[cols].reshape(1, DQ), (P, 1))).astype(np.float32),
            "maskb": np.ascontiguousarray(mb2),
        })
    return in_maps


def kernel(X, mask, Wq, bq, Wk, bk, Wv, bv, Wo, bo):
    bo = np.asarray(bo, dtype=np.float32)
    nc = _get_program()
    in_maps = _make_in_maps(dict(X=X, mask=mask, Wq=Wq, bq=bq, Wk=Wk, bk=bk,
                                 Wv=Wv, bv=bv, Wo=Wo, bo=bo))
    res = run_bass_kernel_spmd(nc, in_maps, list(range(NCORES))).results
    out = np.zeros((B, S, H), dtype=np.float32)
    for c in range(NCORES):
        out[c // 4] += res[c]["part"]
    out += bo
    return out


# revision 95
# speedup vs baseline: 1.0816x; 1.0087x over previous
"""Multi-head attention (B=2, S=2048, H=1024, 16 heads x 64) on 8 NeuronCores.

Sharding: tensor-parallel over heads x data-parallel over batch.
Core c handles batch (c // 4) and heads [4*(c%4), 4*(c%4)+4).
Each core computes its 4 heads' QKV projections, attention, and the partial
output projection ctx_h @ Wo_h; the host sums the 4 partials per batch.

The datapath is fp16 (noise ~5e-4; fp8 was tried and its ~2.5%/stage
quantization noise transfers 1:1 through the softmax-weighted mean, far
over the accuracy budget). fp16 matmuls run at the same 1 cycle/row as
fp32r but with half the SBUF/DMA traffic. Structural savings vs the fp32
baseline:
 - V is computed directly in [t, dv] layout by making X the stationary
   matmul operand, eliminating all PE transposes and their drains.
 - The output projection packs the two heads of a pair on the contraction
   dim (K=128 instead of 64), halving its PE time. For the first half of
   the sequence it runs as two passes overlapped with late attention
   (pair 0 during h2/h3, pair 1 as h3 filler); the second half runs
   single-pass at the end with drains alternating DVE/ACT.
 - exp outputs fp16 directly (with a -4 global shift so e^score stays in
   range; the shift cancels in the softmax ratio), halving e-tile traffic.
Softmax skips max-subtraction and gets its denominator for free from an
appended ones-column on V; 1/den is broadcast over dv rows with a K=1 PE
outer product (no DRAM round trip). ctx runs 4 t-tiles behind exp so the
in-order PE never waits on ACT latency; projections drip in as
single-matmul filler sub-tasks whose emission order respects each
consumer's deadline (the tile framework only syncs in emission order).
"""
import numpy as np

import concourse.bass as bass
import concourse.tile as tile
from concourse import bacc, mybir
from concourse.bass_utils import run_bass_kernel_spmd

F32 = mybir.dt.float32
F32R = mybir.dt.float32r
F16 = mybir.dt.float16

H, NH, HD = 1024, 16, 64
B, S = 2, 2048
P = 128
NCORES = 8
NHL = 4          # heads per core
DQ = NHL * HD    # 256 projection cols per core
NHT = H // P     # 8 h-tiles
NST = S // P     # 16 t-tiles (also s-tiles)
SB = 512         # matmul free-dim block
SS = 1024        # attention s-superblock (2 PSUM banks)
NSB = S // SB    # 4
NSS = S // SS    # 2

EXP_SHIFT = -4.0  # global exp shift (cancels in softmax); keeps e^score
                  # well inside fp16 range for scores up to ~14


def _reshape_free(ap, dims):
    """Reinterpret a contiguous free region of `ap` as `dims`."""
    total = 1
    new = []
    for d in reversed(dims):
        new.append([total, d])
        total *= d
    assert total == ap.free_size()
    return bass.AP(tensor=ap.tensor, offset=ap.offset,
                   ap=[ap.ap[0]] + list(reversed(new)))


def build_program(repeat=1):
    nc = bacc.Bacc("TRN2", target_bir_lowering=False, debug=False,
                   num_devices=NCORES)
    _lp = nc.allow_low_precision(reason="fp16 attention pipeline")
    _lp.__enter__()

    xt_d = nc.dram_tensor("xt", [H, S], F16, kind="ExternalInput").ap()
    wq_d = nc.dram_tensor("wq", [H, DQ], F16, kind="ExternalInput").ap()
    wk_d = nc.dram_tensor("wk", [H, DQ], F16, kind="ExternalInput").ap()
    wv_d = nc.dram_tensor("wv", [H, DQ], F16, kind="ExternalInput").ap()
    wo_d = nc.dram_tensor("wo", [P, 2, H], F32R, kind="ExternalInput").ap()
    bq_d = nc.dram_tensor("bq", [P, 2], F32, kind="ExternalInput").ap()
    bk_d = nc.dram_tensor("bk", [P, 2], F32, kind="ExternalInput").ap()
    bvb_d = nc.dram_tensor("bvb", [P, DQ], F32, kind="ExternalInput").ap()
    mb_d = nc.dram_tensor("maskb", [P, NST], F32, kind="ExternalInput").ap()
    part_d = nc.dram_tensor("part", [S, H], F16, kind="ExternalOutput").ap()

    with tile.TileContext(nc) as tc:
        with tc.tile_pool(name="big", bufs=1) as big, \
             tc.tile_pool(name="consts", bufs=1) as consts, \
             tc.tile_pool(name="epool", bufs=6) as epool, \
             tc.tile_pool(name="bcpool", bufs=2) as bcpool, \
             tc.tile_pool(name="opool", bufs=1) as opool, \
             tc.tile_pool(name="dpool", bufs=2) as dpool, \
             tc.tile_pool(name="ps_sc", bufs=2, space="PSUM") as ps_sc, \
             tc.tile_pool(name="ps_ctx", bufs=1, space="PSUM") as ps_ctx, \
             tc.tile_pool(name="ps_mm", bufs=2, space="PSUM") as ps_mm:

            for _it in range(repeat):
                # ---------------- input loads ----------------
                xt_sb = big.tile([P, NHT, S], F16, tag="xt", name="xt_sb")
                xt_r = xt_d.rearrange("(n p) s -> n p s", p=P)
                wq_sb = consts.tile([P, NHT, DQ], F16, tag="wq", name="wq_sb")
                wk_sb = consts.tile([P, NHT, DQ], F16, tag="wk", name="wk_sb")
                wv_sb = consts.tile([P, NHT, DQ], F16, tag="wv", name="wv_sb")

                xt_rp = xt_d.rearrange("(n p) s -> p n s", p=P)

                def load_x_cols(c0, c1):
                    nc.sync.dma_start(
                        out=xt_sb[:, :, c0:c1], in_=xt_rp[:, :, c0:c1])

                def load_w(w_sb, w_d):
                    nc.sync.dma_start(
                        out=w_sb, in_=w_d.rearrange("(n p) d -> p n d", p=P))

                # wk split in half so the first K matmuls (h-tiles 0-3)
                # start one transfer earlier
                wk_r = wk_d.rearrange("(n p) d -> p n d", p=P)
                nc.sync.dma_start(out=wk_sb[:, 0:4, :], in_=wk_r[:, 0:4, :])
                load_x_cols(0, 256)
                load_x_cols(256, 512)
                nc.sync.dma_start(out=wk_sb[:, 4:8, :], in_=wk_r[:, 4:8, :])
                load_w(wv_sb, wv_d)
                load_w(wq_sb, wq_d)
                load_x_cols(512, 1024)
                load_x_cols(1024, 1536)
                load_x_cols(1536, 2048)

                # tiny tensors ride the idle gpsimd queue so they land in
                # the first few us instead of behind the X stream (the
                # first exp needs mb, the first drains need bq/bk/bvb)
                bq_sb = consts.tile([P, 2], F32, tag="bq", name="bq_sb")
                bk_sb = consts.tile([P, 2], F32, tag="bk", name="bk_sb")
                nc.gpsimd.dma_start(out=bq_sb, in_=bq_d)
                nc.gpsimd.dma_start(out=bk_sb, in_=bk_d)
                mb_sb = consts.tile([P, NST], F32, tag="mb", name="mb_sb")
                nc.gpsimd.dma_start(out=mb_sb, in_=mb_d)
                bvb_sb = consts.tile([P, DQ], F32, tag="bvb", name="bvb_sb")
                nc.gpsimd.dma_start(out=bvb_sb, in_=bvb_d)
                wo_sb = consts.tile([P, 2, H], F32R, tag="wo", name="wo_sb")
                nc.sync.dma_start(out=wo_sb, in_=wo_d)

                # projection outputs: Q^T/K^T in [dv(2 heads), pair, s]
                qT = big.tile([P, 2, S], F16, tag="qT", name="qT")
                kT = big.tile([P, 2, S], F16, tag="kT", name="kT")
                # V (+ones col) in [t, st, head, dv] layout
                vaug = big.tile([P, NST, NHL, HD + 1], F16, tag="vaug",
                                name="vaug")
                nc.vector.memset(vaug[:, :, :, HD:HD + 1], 1.0)

                ctx2 = [big.tile([P, S], F32R, tag=f"ctx2_{pr}",
                                 name=f"ctx2_{pr}") for pr in range(2)]

                rec_rows = {}
                ones128 = consts.tile([1, P], F32R, tag="ones128",
                                      name="ones128")
                one = nc.const_aps.aps[(F32, 1.0)]
                ones_src = bass.AP(tensor=one.tensor, offset=one.offset,
                                   ap=[[one.ap[0][0], 1], [0, P]])
                nc.vector.tensor_copy(ones128, ones_src)

                # dummy exp to pull the ACT Exp-table load (1.3us) into the
                # DMA-bound lead instead of the first real exp's critical path
                warm = consts.tile([1, 1], F16, tag="warm", name="warm")
                nc.scalar.activation(out=warm, in_=ones128[0:1, 0:1],
                                     func=mybir.ActivationFunctionType.Exp,
                                     bias=0.0, scale=1.0)

                # ---------------- projection tasks ----------------
                # emitted as single-matmul sub-tasks (~0.2us each) so filler
                # pops never stall the exp-paced attention pipeline
                def qk_subs(dqt, projs="qk", sbs=tuple(range(NSB))):
                    sel = {"q": (wq_sb, bq_sb, qT, "q"),
                           "k": (wk_sb, bk_sb, kT, "k")}
                    subs = []
                    for sb_i in sbs:
                        for w_sb, b_sb, out_sb, nm in (sel[p] for p in projs):
                            st8 = {}

                            def mm(ht, w_sb=w_sb, sb_i=sb_i, st8=st8, nm=nm):
                                def t():
                                    if ht == 0:
                                        st8["acc"] = ps_mm.tile(
                                            [P, SB], F32, tag="mm512",
                                            name=f"acc_{nm}{dqt}_{sb_i}")
                                    nc.tensor.matmul(
                                        st8["acc"],
                                        w_sb[:, ht, dqt * P:(dqt + 1) * P],
                                        xt_sb[:, ht,
                                              sb_i * SB:(sb_i + 1) * SB],
                                        start=(ht == 0), stop=(ht == NHT - 1))
                                return t

                            def drain(b_sb=b_sb, out_sb=out_sb, sb_i=sb_i,
                                      st8=st8):
                                def t():
                                    nc.vector.tensor_scalar_add(
                                        out_sb[:, dqt,
                                               sb_i * SB:(sb_i + 1) * SB],
                                        st8["acc"], b_sb[:, dqt:dqt + 1])
                                return t

                            subs += [mm(ht) for ht in range(NHT)]
                            subs.append(drain())
                    return subs

                def v_subs(dqt, sts=tuple(range(NST))):
                    subs = []
                    for st in sts:
                        st8 = {}

                        def mm(ht, st=st, st8=st8):
                            def t():
                                if ht == 0:
                                    st8["acc"] = ps_mm.tile(
                                        [P, SB], F32, tag="mm512",
                                        name=f"vacc{dqt}_{st}")
                                nc.tensor.matmul(
                                    st8["acc"][:, 0:P],
                                    xt_sb[:, ht, st * P:(st + 1) * P],
                                    wv_sb[:, ht, dqt * P:(dqt + 1) * P],
                                    start=(ht == 0), stop=(ht == NHT - 1))
                            return t

                        def drain(st=st, st8=st8):
                            def t():
                                nc.vector.tensor_add(
                                    vaug[:, st, 2 * dqt:2 * dqt + 2, 0:HD],
                                    _reshape_free(st8["acc"][:, 0:P], [2, HD]),
                                    _reshape_free(
                                        bvb_sb[:, dqt * P:(dqt + 1) * P],
                                        [2, HD]))
                            return t

                        subs += [mm(ht) for ht in range(NHT)]
                        subs.append(drain())
                    return subs

                # ---------------- attention ----------------
                def attention(h, filler, rate=2.0, mid=None):
                    base = HD * (h % 2)
                    dvt = h // 2
                    pr = h // 2
                    row = HD * (h % 2)
                    budget = 0.0
                    rates = rate if isinstance(rate, tuple) else (rate, rate)
                    for ssb in range(NSS):
                        rate = rates[ssb]
                        if ssb == 1 and mid is not None:
                            mid()
                        acc = ps_ctx.tile([HD + 1, SS], F32, tag="ctxps",
                                          name=f"ctx_{h}_{ssb}")
                        es = {}
                        # ctx runs TWO t-tiles behind exp so the PE (in-order)
                        # never waits on the ACT exp latency or its semaphore
                        LAG = 4
                        for tt in range(NST + LAG):
                            budget += rate
                            while filler and budget >= 1.0:
                                filler.pop(0)()
                                budget -= 1.0
                            if tt < NST:
                                sc = ps_sc.tile([P, SS], F32, tag="sc",
                                                name=f"sc_{h}_{ssb}_{tt}")
                                for half in range(2):
                                    sb_i = 2 * ssb + half
                                    nc.tensor.matmul(
                                        sc[:, half * SB:(half + 1) * SB],
                                        kT[base:base + HD, dvt,
                                           tt * P:(tt + 1) * P],
                                        qT[base:base + HD, dvt,
                                           sb_i * SB:(sb_i + 1) * SB],
                                        start=True, stop=True)
                            if tt >= LAG:
                                e_in = es.pop(tt - LAG)
                                for half in range(2):
                                    nc.tensor.matmul(
                                        acc[:, half * SB:(half + 1) * SB],
                                        vaug[:, tt - LAG, h, :],
                                        e_in[:, half * SB:(half + 1) * SB],
                                        start=(tt == LAG),
                                        stop=(tt == NST + LAG - 1))
                            if tt < NST:
                                e = epool.tile([P, SS], F16, tag="e",
                                               name=f"e_{h}_{ssb}_{tt}")
                                nc.scalar.activation(
                                    out=e, in_=sc,
                                    func=mybir.ActivationFunctionType.Exp,
                                    bias=mb_sb[:, tt:tt + 1], scale=1.0 / 8.0)
                                es[tt] = e
                        # denominator reciprocals FIRST (the PE's broadcast
                        # matmul waits only on these, not the drains), then
                        # drain ctx; all on-chip. The kernel's last
                        # superblock keeps per-half recips so the tail can
                        # start after half a row.
                        rec_row = dpool.tile([1, SS], F32R, tag="recrow",
                                             name=f"recrow_{h}_{ssb}")
                        if h == NHL - 1 and ssb == NSS - 1:
                            nc.vector.reciprocal(rec_row[:, 0:SB],
                                                 acc[HD:HD + 1, 0:SB])
                            nc.vector.reciprocal(rec_row[:, SB:SS],
                                                 acc[HD:HD + 1, SB:SS])
                        else:
                            nc.vector.reciprocal(rec_row, acc[HD:HD + 1, :])
                        rec_rows[(h, ssb)] = rec_row
                        nc.vector.tensor_copy(
                            ctx2[pr][row:row + HD,
                                     ssb * SS:(ssb + 1) * SS],
                            acc[0:HD, :])

                def rec_thunks(h, ssbs=(0, 1), halves=(0, 1)):
                    # one thunk per (ssb, half) so the normalization can be
                    # dripped as filler into the NEXT head's attention
                    # instead of blocking its scores in the in-order PE queue
                    def mk(ssb, half):
                        def t():
                            rec_one(h, ssb, half)
                        return t
                    return [mk(ssb, half) for ssb in ssbs for half in halves]

                def rec_chain(h, ssbs=(0, 1), halves=(0, 1)):
                    for ssb in ssbs:
                        for half in halves:
                            rec_one(h, ssb, half)

                def rec_one(h, ssb, half):
                    # broadcast 1/den over the dv rows with a K=1 PE outer
                    # product (ones128 x rec_row) and scale ctx2 in place --
                    # fully on-chip, no DRAM round trip
                    pr = h // 2
                    row = HD * (h % 2)
                    if True:
                        rr = rec_rows[(h, ssb)]
                        if True:
                            sb_i = 2 * ssb + half
                            bc = ps_mm.tile([P, SB], F32, tag="mm512",
                                            name=f"bc_{h}_{sb_i}")
                            nc.tensor.matmul(
                                bc, ones128,
                                rr[:, half * SB:(half + 1) * SB],
                                start=True, stop=True)
                            nc.vector.tensor_mul(
                                ctx2[pr][row:row + HD,
                                         sb_i * SB:(sb_i + 1) * SB],
                                ctx2[pr][row:row + HD,
                                         sb_i * SB:(sb_i + 1) * SB],
                                bc[row:row + HD, :])

                # ---------------- output projection ----------------
                o_st = [None] * NST

                def outproj_p0(st, j):
                    def t():
                        if j == 0:
                            o_st[st] = opool.tile([P, H], F16, tag=f"o_{st}",
                                                  name=f"o_{st}")
                        o = o_st[st]
                        po = ps_mm.tile([P, SB], F32, tag="mm512",
                                        name=f"po0_{st}_{j}")
                        nc.tensor.matmul(
                            po,
                            ctx2[0][:, st * P:(st + 1) * P],
                            wo_sb[:, 0, j * SB:(j + 1) * SB],
                            start=True, stop=True)
                        nc.vector.tensor_copy(o[:, j * SB:(j + 1) * SB], po)
                    return t

                def outproj_p1(st):
                    def t():
                        o = o_st[st]
                        for j in range(2):
                            po = ps_mm.tile([P, SB], F32, tag="mm512",
                                            name=f"po1_{st}_{j}")
                            nc.tensor.matmul(
                                po,
                                ctx2[1][:, st * P:(st + 1) * P],
                                wo_sb[:, 1, j * SB:(j + 1) * SB],
                                start=True, stop=True)
                            nc.vector.tensor_add(
                                o[:, j * SB:(j + 1) * SB],
                                po, o[:, j * SB:(j + 1) * SB])
                        nc.sync.dma_start(
                            out=part_d[st * P:(st + 1) * P, :], in_=o)
                    return t

                def outproj(st, use_act):
                    # single pass over both head pairs; at the kernel tail
                    # the drains alternate DVE / ACT so neither paces it,
                    # and po tiles alternate ps_mm / the (now idle) score
                    # pool so PSUM rotation latency doesn't pace it either
                    def t():
                        o = opool.tile([P, H], F16, tag=f"o_{st}",
                                       name=f"o_{st}")
                        for j in range(2):
                            if use_act and j % 2 == 1:
                                po = ps_sc.tile([P, SS], F32, tag="sc",
                                                name=f"po_{st}_{j}")[:, 0:SB]
                            else:
                                po = ps_mm.tile([P, SB], F32, tag="mm512",
                                                name=f"po_{st}_{j}")
                            for pr in range(2):
                                nc.tensor.matmul(
                                    po,
                                    ctx2[pr][:, st * P:(st + 1) * P],
                                    wo_sb[:, pr, j * SB:(j + 1) * SB],
                                    start=(pr == 0), stop=(pr == 1))
                            if use_act and j % 2 == 1:
                                nc.scalar.copy(o[:, j * SB:(j + 1) * SB], po)
                            else:
                                nc.vector.tensor_copy(
                                    o[:, j * SB:(j + 1) * SB], po)
                        nc.sync.dma_start(
                            out=part_d[st * P:(st + 1) * P, :], in_=o)
                    return t

                # ---------------- schedule ----------------
                # inline lead: only what h0's first steps strictly need
                # (K0/Q0 for s,t < 512-1024, V pair-0 tiles 0-3); the rest
                # drips as deadline-ordered fillers during h0-ssb0
                for t in (qk_subs(0, "k", (0,)) + v_subs(0, (0, 1, 2, 3))
                          + qk_subs(0, "q", (0, 1))):
                    t()
                # deadline-ordered h0-ssb0 fillers at 9 pops/step: K0-sb_i
                # EMITTED by step 4i, v0_st by step st (emission order is
                # what guarantees readers see written tiles)
                fill = (qk_subs(0, "k", (1,)) + v_subs(0, (4, 5))
                        + qk_subs(0, "k", (2,)) + v_subs(0, (6, 7, 8))
                        + qk_subs(0, "k", (3,))
                        + v_subs(0, (9, 10, 11, 12, 13, 14, 15))
                        + qk_subs(0, "q", (2, 3))
                        + v_subs(1)
                        + qk_subs(1, "k") + qk_subs(1, "q", (0, 1)))
                attention(0, fill, rate=(9.0, 3.2))
                fill[0:0] = rec_thunks(0)
                attention(1, fill, rate=3.2)
                while fill:
                    fill.pop(0)()
                fill2 = (rec_thunks(1) + qk_subs(1, "q", (2, 3)) + [
                    outproj_p0(st, j) for st in range(NST // 2)
                    for j in range(2)])
                attention(2, fill2, rate=1.2)
                fill2[0:0] = rec_thunks(2)

                def h3_mid():
                    # after h3's first superblock: normalize its s<1024 rows,
                    # then finish the first-half output projection as filler
                    while fill2:
                        fill2.pop(0)()
                    rec_chain(3, ssbs=(0,))
                    fill2.extend(outproj_p1(st) for st in range(NST // 2))

                attention(3, fill2, rate=1.0, mid=h3_mid)
                while fill2:
                    fill2.pop(0)()
                # per-half tail: outproj for s in [1024,1536) starts right
                # after the first half-reciprocal; the second half's
                # normalization overlaps it
                rec_chain(3, ssbs=(1,), halves=(0,))
                outproj(8, True)()
                rec_chain(3, ssbs=(1,), halves=(1,))
                for st in range(9, NST):
                    outproj(st, True)()

    nc.compile()
    return nc


_CACHE = {}


def _get_program(repeat=1):
    key = repeat
    if key not in _CACHE:
        _CACHE[key] = build_program(repeat)
    return _CACHE[key]


def _make_in_maps(inputs):
    X = np.asarray(inputs["X"], dtype=np.float32)
    mask = np.asarray(inputs["mask"], dtype=np.float32)
    Wq = np.asarray(inputs["Wq"], dtype=np.float32)
    Wk = np.asarray(inputs["Wk"], dtype=np.float32)
    Wv = np.asarray(inputs["Wv"], dtype=np.float32)
    Wo = np.asarray(inputs["Wo"], dtype=np.float32)
    bq = np.asarray(inputs["bq"], dtype=np.float32)
    bk = np.asarray(inputs["bk"], dtype=np.float32)
    bv = np.asarray(inputs["bv"], dtype=np.float32)

    f16 = np.float16
    in_maps = []
    xts = [np.ascontiguousarray(X[b].T).astype(f16) for b in range(B)]
    maskbs = [np.ascontiguousarray(-1e6 * (1.0 - mask[b])) for b in range(B)]
    for c in range(NCORES):
        b = c // 4
        g = c % 4
        cols = slice(g * DQ, (g + 1) * DQ)
        mb2 = (maskbs[b].reshape(NST, P).T + EXP_SHIFT).astype(np.float32)
        wo2 = Wo[cols, :].reshape(2, P, H).transpose(1, 0, 2)
        in_maps.append({
            "xt": xts[b],
            "wq": np.ascontiguousarray(Wq[:, cols]).astype(f16),
            "wk": np.ascontiguousarray(Wk[:, cols]).astype(f16),
            "wv": np.ascontiguousarray(Wv[:, cols]).astype(f16),
            "wo": np.ascontiguousarray(wo2),
            "bq": np.ascontiguousarray(bq[cols].reshape(2, P).T),
            "bk": np.ascontiguousarray(bk[cols].reshape(2, P).T),
            "bvb": np.ascontiguousarray(
                np.tile(bv[cols].reshape(1, DQ), (P, 1))).astype(np.float32),
            "maskb": np.ascontiguousarray(mb2),
        })
    return in_maps


def kernel(X, mask, Wq, bq, Wk, bk, Wv, bv, Wo, bo):
    bo = np.asarray(bo, dtype=np.float32)
    nc = _get_program()
    in_maps = _make_in_maps(dict(X=X, mask=mask, Wq=Wq, bq=bq, Wk=Wk, bk=bk,
                                 Wv=Wv, bv=bv, Wo=Wo, bo=bo))
    res = run_bass_kernel_spmd(nc, in_maps, list(range(NCORES))).results
    out = np.zeros((B, S, H), dtype=np.float32)
    for c in range(NCORES):
        out[c // 4] += res[c]["part"]
    out += bo
    return out


# revision 96
# speedup vs baseline: 1.0903x; 1.0081x over previous
"""Multi-head attention (B=2, S=2048, H=1024, 16 heads x 64) on 8 NeuronCores.

Sharding: tensor-parallel over heads x data-parallel over batch.
Core c handles batch (c // 4) and heads [4*(c%4), 4*(c%4)+4).
Each core computes its 4 heads' QKV projections, attention, and the partial
output projection ctx_h @ Wo_h; the host sums the 4 partials per batch.

The datapath is fp16 (noise ~5e-4; fp8 was tried and its ~2.5%/stage
quantization noise transfers 1:1 through the softmax-weighted mean, far
over the accuracy budget). fp16 matmuls run at the same 1 cycle/row as
fp32r but with half the SBUF/DMA traffic. Structural savings vs the fp32
baseline:
 - V is computed directly in [t, dv] layout by making X the stationary
   matmul operand, eliminating all PE transposes and their drains.
 - The output projection packs the two heads of a pair on the contraction
   dim (K=128 instead of 64), halving its PE time. For the first half of
   the sequence it runs as two passes overlapped with late attention
   (pair 0 during h2/h3, pair 1 as h3 filler); the second half runs
   single-pass at the end with drains alternating DVE/ACT.
 - exp outputs fp16 directly (with a -4 global shift so e^score stays in
   range; the shift cancels in the softmax ratio), halving e-tile traffic.
Softmax skips max-subtraction and gets its denominator for free from an
appended ones-column on V; 1/den is broadcast over dv rows with a K=1 PE
outer product (no DRAM round trip). ctx runs 4 t-tiles behind exp so the
in-order PE never waits on ACT latency; projections drip in as
single-matmul filler sub-tasks whose emission order respects each
consumer's deadline (the tile framework only syncs in emission order).
"""
import numpy as np

import concourse.bass as bass
import concourse.tile as tile
from concourse import bacc, mybir
from concourse.bass_utils import run_bass_kernel_spmd

F32 = mybir.dt.float32
F32R = mybir.dt.float32r
F16 = mybir.dt.float16

H, NH, HD = 1024, 16, 64
B, S = 2, 2048
P = 128
NCORES = 8
NHL = 4          # heads per core
DQ = NHL * HD    # 256 projection cols per core
NHT = H // P     # 8 h-tiles
NST = S // P     # 16 t-tiles (also s-tiles)
SB = 512         # matmul free-dim block
SS = 1024        # attention s-superblock (2 PSUM banks)
NSB = S // SB    # 4
NSS = S // SS    # 2

EXP_SHIFT = -4.0  # global exp shift (cancels in softmax); keeps e^score
                  # well inside fp16 range for scores up to ~14


def _reshape_free(ap, dims):
    """Reinterpret a contiguous free region of `ap` as `dims`."""
    total = 1
    new = []
    for d in reversed(dims):
        new.append([total, d])
        total *= d
    assert total == ap.free_size()
    return bass.AP(tensor=ap.tensor, offset=ap.offset,
                   ap=[ap.ap[0]] + list(reversed(new)))


def build_program(repeat=1):
    nc = bacc.Bacc("TRN2", target_bir_lowering=False, debug=False,
                   num_devices=NCORES)
    _lp = nc.allow_low_precision(reason="fp16 attention pipeline")
    _lp.__enter__()

    xt_d = nc.dram_tensor("xt", [H, S], F16, kind="ExternalInput").ap()
    wq_d = nc.dram_tensor("wq", [H, DQ], F16, kind="ExternalInput").ap()
    wk_d = nc.dram_tensor("wk", [H, DQ], F16, kind="ExternalInput").ap()
    wv_d = nc.dram_tensor("wv", [H, DQ], F16, kind="ExternalInput").ap()
    wo_d = nc.dram_tensor("wo", [P, 2, H], F32R, kind="ExternalInput").ap()
    bq_d = nc.dram_tensor("bq", [P, 2], F32, kind="ExternalInput").ap()
    bk_d = nc.dram_tensor("bk", [P, 2], F32, kind="ExternalInput").ap()
    bvb_d = nc.dram_tensor("bvb", [P, DQ], F32, kind="ExternalInput").ap()
    mb_d = nc.dram_tensor("maskb", [P, NST], F32, kind="ExternalInput").ap()
    part_d = nc.dram_tensor("part", [S, H], F16, kind="ExternalOutput").ap()

    with tile.TileContext(nc) as tc:
        with tc.tile_pool(name="big", bufs=1) as big, \
             tc.tile_pool(name="consts", bufs=1) as consts, \
             tc.tile_pool(name="epool", bufs=6) as epool, \
             tc.tile_pool(name="bcpool", bufs=2) as bcpool, \
             tc.tile_pool(name="opool", bufs=1) as opool, \
             tc.tile_pool(name="dpool", bufs=2) as dpool, \
             tc.tile_pool(name="ps_sc", bufs=2, space="PSUM") as ps_sc, \
             tc.tile_pool(name="ps_ctx", bufs=1, space="PSUM") as ps_ctx, \
             tc.tile_pool(name="ps_mm", bufs=2, space="PSUM") as ps_mm:

            for _it in range(repeat):
                # ---------------- input loads ----------------
                xt_sb = big.tile([P, NHT, S], F16, tag="xt", name="xt_sb")
                xt_r = xt_d.rearrange("(n p) s -> n p s", p=P)
                wq_sb = consts.tile([P, NHT, DQ], F16, tag="wq", name="wq_sb")
                wk_sb = consts.tile([P, NHT, DQ], F16, tag="wk", name="wk_sb")
                wv_sb = consts.tile([P, NHT, DQ], F16, tag="wv", name="wv_sb")

                xt_rp = xt_d.rearrange("(n p) s -> p n s", p=P)

                def load_x_cols(c0, c1):
                    nc.sync.dma_start(
                        out=xt_sb[:, :, c0:c1], in_=xt_rp[:, :, c0:c1])

                def load_w(w_sb, w_d):
                    nc.sync.dma_start(
                        out=w_sb, in_=w_d.rearrange("(n p) d -> p n d", p=P))

                # wk split in half so the first K matmuls (h-tiles 0-3)
                # start one transfer earlier
                wk_r = wk_d.rearrange("(n p) d -> p n d", p=P)
                nc.sync.dma_start(out=wk_sb[:, 0:4, :], in_=wk_r[:, 0:4, :])
                nc.sync.dma_start(out=xt_sb[:, 0:4, 0:512],
                                  in_=xt_rp[:, 0:4, 0:512])
                nc.sync.dma_start(out=xt_sb[:, 4:8, 0:512],
                                  in_=xt_rp[:, 4:8, 0:512])
                nc.sync.dma_start(out=wk_sb[:, 4:8, :], in_=wk_r[:, 4:8, :])
                load_w(wv_sb, wv_d)
                load_w(wq_sb, wq_d)
                load_x_cols(512, 1024)
                load_x_cols(1024, 1536)
                load_x_cols(1536, 2048)

                # tiny tensors ride the idle gpsimd queue so they land in
                # the first few us instead of behind the X stream (the
                # first exp needs mb, the first drains need bq/bk/bvb)
                bq_sb = consts.tile([P, 2], F32, tag="bq", name="bq_sb")
                bk_sb = consts.tile([P, 2], F32, tag="bk", name="bk_sb")
                nc.gpsimd.dma_start(out=bq_sb, in_=bq_d)
                nc.gpsimd.dma_start(out=bk_sb, in_=bk_d)
                mb_sb = consts.tile([P, NST], F32, tag="mb", name="mb_sb")
                nc.gpsimd.dma_start(out=mb_sb, in_=mb_d)
                bvb_sb = consts.tile([P, DQ], F32, tag="bvb", name="bvb_sb")
                nc.gpsimd.dma_start(out=bvb_sb, in_=bvb_d)
                wo_sb = consts.tile([P, 2, H], F32R, tag="wo", name="wo_sb")
                nc.sync.dma_start(out=wo_sb, in_=wo_d)

                # projection outputs: Q^T/K^T in [dv(2 heads), pair, s]
                qT = big.tile([P, 2, S], F16, tag="qT", name="qT")
                kT = big.tile([P, 2, S], F16, tag="kT", name="kT")
                # V (+ones col) in [t, st, head, dv] layout
                vaug = big.tile([P, NST, NHL, HD + 1], F16, tag="vaug",
                                name="vaug")
                nc.vector.memset(vaug[:, :, :, HD:HD + 1], 1.0)

                ctx2 = [big.tile([P, S], F32R, tag=f"ctx2_{pr}",
                                 name=f"ctx2_{pr}") for pr in range(2)]

                rec_rows = {}
                ones128 = consts.tile([1, P], F32R, tag="ones128",
                                      name="ones128")
                one = nc.const_aps.aps[(F32, 1.0)]
                ones_src = bass.AP(tensor=one.tensor, offset=one.offset,
                                   ap=[[one.ap[0][0], 1], [0, P]])
                nc.vector.tensor_copy(ones128, ones_src)

                # dummy exp to pull the ACT Exp-table load (1.3us) into the
                # DMA-bound lead instead of the first real exp's critical path
                warm = consts.tile([1, 1], F16, tag="warm", name="warm")
                nc.scalar.activation(out=warm, in_=ones128[0:1, 0:1],
                                     func=mybir.ActivationFunctionType.Exp,
                                     bias=0.0, scale=1.0)

                # ---------------- projection tasks ----------------
                # emitted as single-matmul sub-tasks (~0.2us each) so filler
                # pops never stall the exp-paced attention pipeline
                def qk_subs(dqt, projs="qk", sbs=tuple(range(NSB))):
                    sel = {"q": (wq_sb, bq_sb, qT, "q"),
                           "k": (wk_sb, bk_sb, kT, "k")}
                    subs = []
                    for sb_i in sbs:
                        for w_sb, b_sb, out_sb, nm in (sel[p] for p in projs):
                            st8 = {}

                            def mm(ht, w_sb=w_sb, sb_i=sb_i, st8=st8, nm=nm):
                                def t():
                                    if ht == 0:
                                        st8["acc"] = ps_mm.tile(
                                            [P, SB], F32, tag="mm512",
                                            name=f"acc_{nm}{dqt}_{sb_i}")
                                    nc.tensor.matmul(
                                        st8["acc"],
                                        w_sb[:, ht, dqt * P:(dqt + 1) * P],
                                        xt_sb[:, ht,
                                              sb_i * SB:(sb_i + 1) * SB],
                                        start=(ht == 0), stop=(ht == NHT - 1))
                                return t

                            def drain(b_sb=b_sb, out_sb=out_sb, sb_i=sb_i,
                                      st8=st8):
                                def t():
                                    nc.vector.tensor_scalar_add(
                                        out_sb[:, dqt,
                                               sb_i * SB:(sb_i + 1) * SB],
                                        st8["acc"], b_sb[:, dqt:dqt + 1])
                                return t

                            subs += [mm(ht) for ht in range(NHT)]
                            subs.append(drain())
                    return subs

                def v_subs(dqt, sts=tuple(range(NST))):
                    subs = []
                    for st in sts:
                        st8 = {}

                        def mm(ht, st=st, st8=st8):
                            def t():
                                if ht == 0:
                                    st8["acc"] = ps_mm.tile(
                                        [P, SB], F32, tag="mm512",
                                        name=f"vacc{dqt}_{st}")
                                nc.tensor.matmul(
                                    st8["acc"][:, 0:P],
                                    xt_sb[:, ht, st * P:(st + 1) * P],
                                    wv_sb[:, ht, dqt * P:(dqt + 1) * P],
                                    start=(ht == 0), stop=(ht == NHT - 1))
                            return t

                        def drain(st=st, st8=st8):
                            def t():
                                nc.vector.tensor_add(
                                    vaug[:, st, 2 * dqt:2 * dqt + 2, 0:HD],
                                    _reshape_free(st8["acc"][:, 0:P], [2, HD]),
                                    _reshape_free(
                                        bvb_sb[:, dqt * P:(dqt + 1) * P],
                                        [2, HD]))
                            return t

                        subs += [mm(ht) for ht in range(NHT)]
                        subs.append(drain())
                    return subs

                # ---------------- attention ----------------
                def attention(h, filler, rate=2.0, mid=None):
                    base = HD * (h % 2)
                    dvt = h // 2
                    pr = h // 2
                    row = HD * (h % 2)
                    budget = 0.0
                    rates = rate if isinstance(rate, tuple) else (rate, rate)
                    for ssb in range(NSS):
                        rate = rates[ssb]
                        if ssb == 1 and mid is not None:
                            mid()
                        acc = ps_ctx.tile([HD + 1, SS], F32, tag="ctxps",
                                          name=f"ctx_{h}_{ssb}")
                        es = {}
                        # ctx runs TWO t-tiles behind exp so the PE (in-order)
                        # never waits on the ACT exp latency or its semaphore
                        LAG = 4
                        for tt in range(NST + LAG):
                            budget += rate
                            while filler and budget >= 1.0:
                                filler.pop(0)()
                                budget -= 1.0
                            if tt < NST:
                                sc = ps_sc.tile([P, SS], F32, tag="sc",
                                                name=f"sc_{h}_{ssb}_{tt}")
                                for half in range(2):
                                    sb_i = 2 * ssb + half
                                    nc.tensor.matmul(
                                        sc[:, half * SB:(half + 1) * SB],
                                        kT[base:base + HD, dvt,
                                           tt * P:(tt + 1) * P],
                                        qT[base:base + HD, dvt,
                                           sb_i * SB:(sb_i + 1) * SB],
                                        start=True, stop=True)
                            if tt >= LAG:
                                e_in = es.pop(tt - LAG)
                                for half in range(2):
                                    nc.tensor.matmul(
                                        acc[:, half * SB:(half + 1) * SB],
                                        vaug[:, tt - LAG, h, :],
                                        e_in[:, half * SB:(half + 1) * SB],
                                        start=(tt == LAG),
                                        stop=(tt == NST + LAG - 1))
                            if tt < NST:
                                e = epool.tile([P, SS], F16, tag="e",
                                               name=f"e_{h}_{ssb}_{tt}")
                                nc.scalar.activation(
                                    out=e, in_=sc,
                                    func=mybir.ActivationFunctionType.Exp,
                                    bias=mb_sb[:, tt:tt + 1], scale=1.0 / 8.0)
                                es[tt] = e
                        # denominator reciprocals FIRST (the PE's broadcast
                        # matmul waits only on these, not the drains), then
                        # drain ctx; all on-chip. The kernel's last
                        # superblock keeps per-half recips so the tail can
                        # start after half a row.
                        rec_row = dpool.tile([1, SS], F32R, tag="recrow",
                                             name=f"recrow_{h}_{ssb}")
                        if h == NHL - 1 and ssb == NSS - 1:
                            nc.vector.reciprocal(rec_row[:, 0:SB],
                                                 acc[HD:HD + 1, 0:SB])
                            nc.vector.reciprocal(rec_row[:, SB:SS],
                                                 acc[HD:HD + 1, SB:SS])
                        else:
                            nc.vector.reciprocal(rec_row, acc[HD:HD + 1, :])
                        rec_rows[(h, ssb)] = rec_row
                        nc.vector.tensor_copy(
                            ctx2[pr][row:row + HD,
                                     ssb * SS:(ssb + 1) * SS],
                            acc[0:HD, :])

                def rec_thunks(h, ssbs=(0, 1), halves=(0, 1)):
                    # one thunk per (ssb, half) so the normalization can be
                    # dripped as filler into the NEXT head's attention
                    # instead of blocking its scores in the in-order PE queue
                    def mk(ssb, half):
                        def t():
                            rec_one(h, ssb, half)
                        return t
                    return [mk(ssb, half) for ssb in ssbs for half in halves]

                def rec_chain(h, ssbs=(0, 1), halves=(0, 1)):
                    for ssb in ssbs:
                        for half in halves:
                            rec_one(h, ssb, half)

                def rec_one(h, ssb, half):
                    # broadcast 1/den over the dv rows with a K=1 PE outer
                    # product (ones128 x rec_row) and scale ctx2 in place --
                    # fully on-chip, no DRAM round trip
                    pr = h // 2
                    row = HD * (h % 2)
                    if True:
                        rr = rec_rows[(h, ssb)]
                        if True:
                            sb_i = 2 * ssb + half
                            bc = ps_mm.tile([P, SB], F32, tag="mm512",
                                            name=f"bc_{h}_{sb_i}")
                            nc.tensor.matmul(
                                bc, ones128,
                                rr[:, half * SB:(half + 1) * SB],
                                start=True, stop=True)
                            nc.vector.tensor_mul(
                                ctx2[pr][row:row + HD,
                                         sb_i * SB:(sb_i + 1) * SB],
                                ctx2[pr][row:row + HD,
                                         sb_i * SB:(sb_i + 1) * SB],
                                bc[row:row + HD, :])

                # ---------------- output projection ----------------
                o_st = [None] * NST

                def outproj_p0(st, j):
                    def t():
                        if j == 0:
                            o_st[st] = opool.tile([P, H], F16, tag=f"o_{st}",
                                                  name=f"o_{st}")
                        o = o_st[st]
                        po = ps_mm.tile([P, SB], F32, tag="mm512",
                                        name=f"po0_{st}_{j}")
                        nc.tensor.matmul(
                            po,
                            ctx2[0][:, st * P:(st + 1) * P],
                            wo_sb[:, 0, j * SB:(j + 1) * SB],
                            start=True, stop=True)
                        nc.vector.tensor_copy(o[:, j * SB:(j + 1) * SB], po)
                    return t

                def outproj_p1(st):
                    def t():
                        o = o_st[st]
                        for j in range(2):
                            po = ps_mm.tile([P, SB], F32, tag="mm512",
                                            name=f"po1_{st}_{j}")
                            nc.tensor.matmul(
                                po,
                                ctx2[1][:, st * P:(st + 1) * P],
                                wo_sb[:, 1, j * SB:(j + 1) * SB],
                                start=True, stop=True)
                            nc.vector.tensor_add(
                                o[:, j * SB:(j + 1) * SB],
                                po, o[:, j * SB:(j + 1) * SB])
                        nc.sync.dma_start(
                            out=part_d[st * P:(st + 1) * P, :], in_=o)
                    return t

                def outproj(st, use_act):
                    # single pass over both head pairs; at the kernel tail
                    # the drains alternate DVE / ACT so neither paces it,
                    # and po tiles alternate ps_mm / the (now idle) score
                    # pool so PSUM rotation latency doesn't pace it either
                    def t():
                        o = opool.tile([P, H], F16, tag=f"o_{st}",
                                       name=f"o_{st}")
                        for j in range(2):
                            if use_act and j % 2 == 1:
                                po = ps_sc.tile([P, SS], F32, tag="sc",
                                                name=f"po_{st}_{j}")[:, 0:SB]
                            else:
                                po = ps_mm.tile([P, SB], F32, tag="mm512",
                                                name=f"po_{st}_{j}")
                            for pr in range(2):
                                nc.tensor.matmul(
                                    po,
                                    ctx2[pr][:, st * P:(st + 1) * P],
                                    wo_sb[:, pr, j * SB:(j + 1) * SB],
                                    start=(pr == 0), stop=(pr == 1))
                            if use_act and j % 2 == 1:
                                nc.scalar.copy(o[:, j * SB:(j + 1) * SB], po)
                            else:
                                nc.vector.tensor_copy(
                                    o[:, j * SB:(j + 1) * SB], po)
                        nc.sync.dma_start(
                            out=part_d[st * P:(st + 1) * P, :], in_=o)
                    return t

                # ---------------- schedule ----------------
                # inline lead: only what h0's first steps strictly need
                # (K0/Q0 for s,t < 512-1024, V pair-0 tiles 0-3); the rest
                # drips as deadline-ordered fillers during h0-ssb0
                for t in (qk_subs(0, "k", (0,)) + v_subs(0, (0, 1, 2, 3))
                          + qk_subs(0, "q", (0, 1))):
                    t()
                # deadline-ordered h0-ssb0 fillers at 9 pops/step: K0-sb_i
                # EMITTED by step 4i, v0_st by step st (emission order is
                # what guarantees readers see written tiles)
                fill = (qk_subs(0, "k", (1,)) + v_subs(0, (4, 5))
                        + qk_subs(0, "k", (2,)) + v_subs(0, (6, 7, 8))
                        + qk_subs(0, "k", (3,))
                        + v_subs(0, (9, 10, 11, 12, 13, 14, 15))
                        + qk_subs(0, "q", (2, 3))
                        + v_subs(1)
                        + qk_subs(1, "k") + qk_subs(1, "q", (0, 1)))
                attention(0, fill, rate=(9.0, 3.2))
                fill[0:0] = rec_thunks(0)
                attention(1, fill, rate=3.2)
                while fill:
                    fill.pop(0)()
                fill2 = (rec_thunks(1) + qk_subs(1, "q", (2, 3)) + [
                    outproj_p0(st, j) for st in range(NST // 2)
                    for j in range(2)])
                attention(2, fill2, rate=1.2)
                fill2[0:0] = rec_thunks(2)

                def h3_mid():
                    # after h3's first superblock: normalize its s<1024 rows,
                    # then finish the first-half output projection as filler
                    while fill2:
                        fill2.pop(0)()
                    rec_chain(3, ssbs=(0,))
                    fill2.extend(outproj_p1(st) for st in range(NST // 2))

                attention(3, fill2, rate=1.0, mid=h3_mid)
                while fill2:
                    fill2.pop(0)()
                # per-half tail: outproj for s in [1024,1536) starts right
                # after the first half-reciprocal; the second half's
                # normalization overlaps it
                rec_chain(3, ssbs=(1,), halves=(0,))
                outproj(8, True)()
                rec_chain(3, ssbs=(1,), halves=(1,))
                for st in range(9, NST):
                    outproj(st, True)()

    nc.compile()
    return nc


_CACHE = {}


def _get_program(repeat=1):
    key = repeat
    if key not in _CACHE:
        _CACHE[key] = build_program(repeat)
    return _CACHE[key]


def _make_in_maps(inputs):
    X = np.asarray(inputs["X"], dtype=np.float32)
    mask = np.asarray(inputs["mask"], dtype=np.float32)
    Wq = np.asarray(inputs["Wq"], dtype=np.float32)
    Wk = np.asarray(inputs["Wk"], dtype=np.float32)
    Wv = np.asarray(inputs["Wv"], dtype=np.float32)
    Wo = np.asarray(inputs["Wo"], dtype=np.float32)
    bq = np.asarray(inputs["bq"], dtype=np.float32)
    bk = np.asarray(inputs["bk"], dtype=np.float32)
    bv = np.asarray(inputs["bv"], dtype=np.float32)

    f16 = np.float16
    in_maps = []
    xts = [np.ascontiguousarray(X[b].T).astype(f16) for b in range(B)]
    maskbs = [np.ascontiguousarray(-1e6 * (1.0 - mask[b])) for b in range(B)]
    for c in range(NCORES):
        b = c // 4
        g = c % 4
        cols = slice(g * DQ, (g + 1) * DQ)
        mb2 = (maskbs[b].reshape(NST, P).T + EXP_SHIFT).astype(np.float32)
        wo2 = Wo[cols, :].reshape(2, P, H).transpose(1, 0, 2)
        in_maps.append({
            "xt": xts[b],
            "wq": np.ascontiguousarray(Wq[:, cols]).astype(f16),
            "wk": np.ascontiguousarray(Wk[:, cols]).astype(f16),
            "wv": np.ascontiguousarray(Wv[:, cols]).astype(f16),
            "wo": np.ascontiguousarray(wo2),
            "bq": np.ascontiguousarray(bq[cols].reshape(2, P).T),
            "bk": np.ascontiguousarray(bk[cols].reshape(2, P).T),
            "bvb": np.ascontiguousarray(
                np.tile(bv[cols].reshape(1, DQ), (P, 1))).astype(np.float32),
            "maskb": np.ascontiguousarray(mb2),
        })
    return in_maps


def kernel(X, mask, Wq, bq, Wk, bk, Wv, bv, Wo, bo):
    bo = np.asarray(bo, dtype=np.float32)
    nc = _get_program()
    in_maps = _make_in_maps(dict(X=X, mask=mask, Wq=Wq, bq=bq, Wk=Wk, bk=bk,
                                 Wv=Wv, bv=bv, Wo=Wo, bo=bo))
    res = run_bass_kernel_spmd(nc, in_maps, list(range(NCORES))).results
    out = np.zeros((B, S, H), dtype=np.float32)
    for c in range(NCORES):
        out[c // 4] += res[c]["part"]
    out += bo
    return out
